# revision 69
# baseline (speedup 1.0000x reference)
"""Trainium2 Bass kernel for nn_DynamicFiltering.

Computation (per batch b):
  xf = frames of x                     (t, c, h, w)
  y  = LeakyReLU(conv2d(xf, w1, b1), 0.2)
  ker = conv2d(y, w2, b2)              (t, 9, h, w)
  ker = ker - mean_k(ker) + 1/45       (per-pixel kernel over K = t*3*3 = 45)
  out[c,h,w] = sum_{t,k1,k2} x_edge[c,t,h+k1-1,w+k2-1] * ker[t,k1,k2][h,w]

Sharding: 8 cores = 2 batches x 4 H-slabs of 32 rows.

Per-core device program (v2, bf16):
  - conv1/conv2 as bf16 matmuls with K=128 tap pairing: the rhs tiles hold
    the image in partitions 0:64 and a one-row-shifted copy in 64:128, so
    taps (0,dj)+(1,dj) share one matmul and (2,dj) runs on the top half.
    6 matmuls per 4-row chunk instead of 9.
  - LeakyReLU fused into a single scalar-engine Lrelu activation.
  - per-frame kernel transpose to pixel-partition layout via one XBAR DMA
    transpose (16x128 tiles) + one DVE strided copy into [q, tap, r] form.
  - dynamic filtering on DVE in bf16 with every operand innermost-packed
    (xt is [q, c, r], kernel broadcast over c with innermost r) so the
    16-bit 2x mode applies; bf16 accumulators, one per dj column shift.
  - dj merge pre-transpose via DMA partition shifts; 16 f32r PE transposes;
    DMA out.
"""

import numpy as np

DIM = 64
T = 5
H = 128
W = 128
SLAB = 32          # output rows per core
NCORES = 8

_PROGRAM_CACHE = {}


def _build_program(debug=False):
    import concourse.bacc as bacc
    import concourse.mybir as mybir
    from concourse.tile import TileContext

    f32 = mybir.dt.float32
    f32r = mybir.dt.float32r
    bf16 = mybir.dt.bfloat16
    Act = mybir.ActivationFunctionType
    Alu = mybir.AluOpType

    nc = bacc.Bacc("TRN2", debug=False)

    xp_d = nc.dram_tensor("xp", [128, T, 36, 130], bf16, kind="ExternalInput").ap()
    xq_d = nc.dram_tensor("xq", [128, T, 36, 130], bf16, kind="ExternalInput").ap()
    xt_d = nc.dram_tensor("xt", [W, T, DIM, 34], bf16, kind="ExternalInput").ap()
    w1p_d = nc.dram_tensor("w1p", [128, 3, DIM], bf16, kind="ExternalInput").ap()
    w1q_d = nc.dram_tensor("w1q", [128, DIM], bf16, kind="ExternalInput").ap()
    w1s2_d = nc.dram_tensor("w1s2", [64, DIM], bf16, kind="ExternalInput").ap()
    w2p_d = nc.dram_tensor("w2p", [128, 3, 9], bf16, kind="ExternalInput").ap()
    w2s_d = nc.dram_tensor("w2s", [64, 3, 9], bf16, kind="ExternalInput").ap()
    b1_d = nc.dram_tensor("b1c", [DIM, 1], f32, kind="ExternalInput").ap()
    b2_d = nc.dram_tensor("b2c", [9, 1], f32, kind="ExternalInput").ap()
    ym_d = nc.dram_tensor("ymask", [128, 2], f32, kind="ExternalInput").ap()
    em_d = nc.dram_tensor("emask", [W, 1], f32, kind="ExternalInput").ap()
    ef_d = nc.dram_tensor("efold", [W, 1], f32, kind="ExternalInput").ap()
    ea_d = nc.dram_tensor("emA", [W, 1], f32, kind="ExternalInput").ap()
    eb_d = nc.dram_tensor("emB", [W, 1], f32, kind="ExternalInput").ap()
    # permutation matrices for the final fused transpose+shift matmuls
    idb_d = nc.dram_tensor("idb", [128, 128], bf16, kind="ExternalInput").ap()
    pdn_d = nc.dram_tensor("pdn", [128, 128], bf16, kind="ExternalInput").ap()
    pup_d = nc.dram_tensor("pup", [128, 128], bf16, kind="ExternalInput").ap()
    out_d = nc.dram_tensor("out", [DIM, SLAB, W], f32, kind="ExternalOutput").ap()
    if debug:
        dbg_y = nc.dram_tensor("dbg_y", [128, 36, 130], bf16,
                               kind="ExternalOutput").ap()
        dbg_ker = nc.dram_tensor("dbg_ker", [16, SLAB, W], bf16,
                                 kind="ExternalOutput").ap()
        dbg_kta = nc.dram_tensor("dbg_kta", [W, SLAB, 16], bf16,
                                 kind="ExternalOutput").ap()
        dbg_kt2 = nc.dram_tensor("dbg_kt2", [W, T, 2, 16, SLAB // 2], bf16,
                                 kind="ExternalOutput").ap()

    with TileContext(nc) as tc:
        with (
            tc.tile_pool(name="consts", bufs=1) as cpool,
            tc.tile_pool(name="xcp", bufs=2) as xcp,
            tc.tile_pool(name="xtp", bufs=3) as xtp,
            tc.tile_pool(name="yp", bufs=2) as yp,
            tc.tile_pool(name="kerp", bufs=2) as kerp,
            tc.tile_pool(name="ktap", bufs=2) as ktap,
            tc.tile_pool(name="ktp", bufs=1) as ktp,
            tc.tile_pool(name="accp", bufs=1) as accp,
            tc.tile_pool(name="stage", bufs=6) as stp,
            tc.tile_pool(name="obp", bufs=3) as obp,
        ):
            # consts are issued on the scalar-engine DGE so the sync DGE can
            # start streaming frame 0's inputs immediately
            w1p_sb = cpool.tile([128, 3, DIM], bf16)
            nc.scalar.dma_start(out=w1p_sb, in_=w1p_d)
            w1q_sb = cpool.tile([128, DIM], bf16)
            nc.scalar.dma_start(out=w1q_sb, in_=w1q_d)
            w1s2_sb = cpool.tile([64, DIM], bf16)
            nc.scalar.dma_start(out=w1s2_sb, in_=w1s2_d)
            w2p_sb = cpool.tile([128, 3, 9], bf16)
            nc.scalar.dma_start(out=w2p_sb, in_=w2p_d)
            w2s_sb = cpool.tile([128, 3, 9], bf16)
            nc.scalar.dma_start(out=w2s_sb[64:128], in_=w2s_d)
            b1_sb = cpool.tile([DIM, 1], f32)
            nc.scalar.dma_start(out=b1_sb, in_=b1_d)
            b2_sb = cpool.tile([9, 1], f32)
            nc.scalar.dma_start(out=b2_sb, in_=b2_d)
            ym_sb = cpool.tile([128, 2], f32)
            nc.scalar.dma_start(out=ym_sb, in_=ym_d)
            em_sb = cpool.tile([W, 1], f32)
            nc.gpsimd.dma_start(out=em_sb, in_=em_d)
            ef_sb = cpool.tile([W, 1], f32)
            nc.gpsimd.dma_start(out=ef_sb, in_=ef_d)
            ea_sb = cpool.tile([W, 1], f32)
            nc.gpsimd.dma_start(out=ea_sb, in_=ea_d)
            eb_sb = cpool.tile([W, 1], f32)
            nc.gpsimd.dma_start(out=eb_sb, in_=eb_d)
            idb_sb = cpool.tile([128, 128], bf16)
            nc.gpsimd.dma_start(out=idb_sb, in_=idb_d)
            pdn_sb = cpool.tile([128, 128], bf16)
            nc.gpsimd.dma_start(out=pdn_sb, in_=pdn_d)
            pup_sb = cpool.tile([128, 128], bf16)
            nc.gpsimd.dma_start(out=pup_sb, in_=pup_d)
            al_sb = cpool.tile([DIM, 1], f32)
            nc.vector.memset(al_sb, 0.2)

            # per-pixel kernels, [q, frame, r-half, tap16, r16] bf16
            # (taps 9..15 unused; r-half-major so per-half DMAs are
            # contiguous)
            kt2 = ktp.tile([W, T, 2, 16, SLAB // 2], bf16)
            kt_p1 = ktp.tile([W, T, 2, 16, SLAB // 2], bf16)
            kt_m1 = ktp.tile([W, T, 2, 16, SLAB // 2], bf16)
            nc.gpsimd.memset(kt_p1[96:128], 0.0)
            nc.gpsimd.memset(kt_m1[0:32], 0.0)
            sum45 = ktp.tile([W, SLAB], f32)
            t45 = ktp.tile([W, SLAB], f32)

            # bf16 accumulators, one per dj; [q, c, r]
            accs = [accp.tile([W, DIM, SLAB], bf16, name=f"acc{dj}")
                    for dj in range(3)]
            ksrc = [kt_p1, kt2, kt_m1]
            u_sb = accp.tile([W, DIM, 34], bf16)

            def emit_loads(f):
                """Prefetch frame f's inputs; the sync DGE queue carries
                only these bulk loads so they stream FIFO ahead of use."""
                xp = xcp.tile([128, 36, 130], bf16, tag="xp")
                nc.sync.dma_start(out=xp, in_=xp_d[:, f])
                xq = xcp.tile([128, 36, 130], bf16, tag="xq")
                nc.sync.dma_start(out=xq, in_=xq_d[:, f])
                xt_f = xtp.tile([W, DIM, 34], bf16, tag="xt")
                nc.sync.dma_start(out=xt_f, in_=xt_d[:, f])
                return xp, xq, xt_f

            def emit_conv1(f, loads, ps1p):
                xp, xq, xt_f = loads
                y2 = yp.tile([128, 36, 130], bf16, tag="y2")
                nc.gpsimd.memset(y2[:, :, 0:1], 0.0)
                nc.gpsimd.memset(y2[:, :, 129:130], 0.0)
                if f == 0:
                    nc.vector.tensor_copy(u_sb, xt_f)
                else:
                    nc.gpsimd.tensor_tensor(u_sb, u_sb, xt_f, Alu.add)

                for rc in range(9):
                    g0 = 1 + 4 * rc
                    nr = 4 if rc < 8 else 2
                    ps = ps1p.tile([DIM, 4, W], f32, tag="ps1")
                    for i, dj in enumerate(range(3)):
                        nc.tensor.matmul(
                            ps[:, :nr, :],
                            lhsT=w1p_sb[:, dj, :],
                            rhs=xp[:, g0 - 1:g0 - 1 + nr, dj:dj + W],
                            start=(i == 0),
                            stop=False,
                        )
                    nc.tensor.matmul(
                        ps[:, :nr, :],
                        lhsT=w1q_sb,
                        rhs=xq[:, g0:g0 + nr, 0:W],
                        start=False,
                        stop=False,
                    )
                    nc.tensor.matmul(
                        ps[:, :nr, :],
                        lhsT=w1s2_sb,
                        rhs=xq[0:64, g0:g0 + nr, 2:2 + W],
                        start=False,
                        stop=True,
                    )
                    nc.scalar.activation(y2[0:64, g0:g0 + nr, 1:129],
                                         ps[:, :nr], Act.Prelu,
                                         bias=b1_sb, scale=1.0, alpha=al_sb)
                    if rc == 0:
                        # conv2 zero-pads rows outside the image: scale the
                        # y rows that fall outside (mask is 0 on edge slabs)
                        nc.vector.tensor_scalar(y2[0:64, 1:2, 1:129],
                                                y2[0:64, 1:2, 1:129],
                                                ym_sb[0:64, 0:1], None,
                                                Alu.mult)
                    if rc == 4:
                        # early half of the row-shifted copy for tap pairing
                        nc.gpsimd.dma_start(out=y2[64:128, 0:18],
                                            in_=y2[0:64, 1:19])
                nc.vector.tensor_scalar(y2[0:64, 34:35, 1:129],
                                        y2[0:64, 34:35, 1:129],
                                        ym_sb[0:64, 1:2], None, Alu.mult)
                nc.gpsimd.dma_start(out=y2[64:128, 18:35],
                                    in_=y2[0:64, 19:36])
                if debug and f == 0:
                    nc.sync.dma_start(out=dbg_y, in_=y2)
                return y2

            HS = SLAB // 2  # half-slab rows

            def emit_conv2_half(f, h, y2, ps2p):
                """conv2 for output rows h*16..h*16+16, ker transpose and
                kernel staging for that half."""
                ker16 = kerp.tile([16, HS, W], bf16, tag="ker16")
                for rc4 in range(4):
                    rc = 4 * h + rc4
                    c0 = 2 + 4 * rc
                    ps2 = ps2p.tile([9, 4, W], f32, tag="ps2")
                    for i, dj in enumerate(range(3)):
                        nc.tensor.matmul(
                            ps2,
                            lhsT=w2p_sb[:, dj, :],
                            rhs=y2[:, c0 - 1:c0 + 3, dj:dj + W],
                            start=(i == 0),
                            stop=False,
                        )
                    for i, dj in enumerate(range(3)):
                        nc.tensor.matmul(
                            ps2,
                            lhsT=w2s_sb[64:128, dj, :],
                            rhs=y2[64:128, c0:c0 + 4, dj:dj + W],
                            start=False,
                            stop=(i == 2),
                        )
                    nc.scalar.activation(
                        ker16[0:9, 4 * rc4:4 * rc4 + 4, :],
                        ps2, Act.Identity, bias=b2_sb, scale=1.0)

                # transpose (tap, r, q) -> (q, r, tap) via the DMA XBAR
                # (xbar block b of 128 cols lands at out[:, b, :]),
                # then repack to (q, tap, r) so filtering reads are
                # innermost-contiguous (enables the DVE 16-bit 2x mode)
                rh = slice(HS * h, HS * h + HS)
                kt_a = ktap.tile([W, HS, 16], bf16, tag="kta")
                nc.scalar.dma_start_transpose(
                    out=kt_a, in_=ker16.rearrange("k r q -> k (r q)"))
                nc.vector.tensor_copy(kt2[:, f, h],
                                      kt_a.rearrange("q r t -> q t r"))
                if debug and f == 0:
                    nc.sync.dma_start(out=dbg_ker[:, rh], in_=ker16)
                    nc.sync.dma_start(out=dbg_kta[:, rh], in_=kt_a)

                # fold W-edge replicate-pad terms into the dj=1 slot
                ktr = kt2[:, f, h, 0:9, :].rearrange(
                    "q (di dj) r -> q di dj r", di=3, dj=3)
                nc.vector.tensor_tensor(ktr[0:1, :, 1, :], ktr[0:1, :, 1, :],
                                        ktr[0:1, :, 0, :], Alu.add)
                nc.vector.scalar_tensor_tensor(
                    out=ktr[96:128, :, 1, :],
                    in0=ktr[96:128, :, 2, :], scalar=em_sb[96:128, :],
                    in1=ktr[96:128, :, 1, :], op0=Alu.mult, op1=Alu.add)

                # partition-shifted kernel copies for the dj column shifts;
                # only the tap slots each copy is read at (dj=0 resp. dj=2),
                # so they don't serialize against the folds
                src_p = kt2[1:128, f, h, 0:9, :].rearrange(
                    "q (di dj) r -> q dj di r", di=3, dj=3)[:, 0]
                dst_p = kt_p1[0:127, f, h, 0:9, :].rearrange(
                    "q (di dj) r -> q dj di r", di=3, dj=3)[:, 0]
                nc.gpsimd.dma_start(out=dst_p, in_=src_p)
                src_m = kt2[0:127, f, h, 0:9, :].rearrange(
                    "q (di dj) r -> q dj di r", di=3, dj=3)[:, 2]
                dst_m = kt_m1[1:128, f, h, 0:9, :].rearrange(
                    "q (di dj) r -> q dj di r", di=3, dj=3)[:, 2]
                nc.gpsimd.dma_start(out=dst_m, in_=src_m)

                # incremental sum of the 45 (folded) kernel taps
                t_out = sum45 if f == 0 else t45
                nc.vector.tensor_reduce(
                    t_out[:, rh],
                    kt2[:, f, h, 0:9, :].rearrange("q t r -> q r t"),
                    axis=mybir.AxisListType.X, op=Alu.add)
                if f > 0:
                    nc.vector.tensor_tensor(sum45[:, rh], sum45[:, rh],
                                            t45[:, rh], Alu.add)

            def _acc_engine(dj):
                # dj=2's accumulate chain runs on gpsimd to unload the DVE
                return nc.gpsimd if dj == 2 else nc.vector

            def emit_filter_half(f, h, xt_f):
                rh = slice(HS * h, HS * h + HS)
                for dj in range(3):
                    eng = _acc_engine(dj)
                    prods = []
                    for di in range(3):
                        kb = ksrc[dj][:, f, h, 3 * di + dj, :].unsqueeze(1)\
                            .broadcast_to((W, DIM, HS))
                        prod = stp.tile([W, DIM, HS], bf16, tag="prod")
                        nc.vector.tensor_tensor(
                            prod, xt_f[:, :, HS * h + di:HS * h + di + HS],
                            kb, Alu.mult)
                        prods.append(prod)
                    a = accs[dj][:, :, rh]
                    if f == 0:
                        eng.tensor_tensor(a, prods[0], prods[1], Alu.add)
                    else:
                        eng.tensor_tensor(a, a, prods[0], Alu.add)
                        eng.tensor_tensor(a, a, prods[1], Alu.add)
                    eng.tensor_tensor(a, a, prods[2], Alu.add)

            def emit_filter_full(f, xt_f):
                # full-slab filtering: the kernel operand spans both r-halves
                # via a 4D view (innermost r stays packed for the 2x mode)
                for dj in range(3):
                    eng = _acc_engine(dj)
                    prods = []
                    for di in range(3):
                        kb = ksrc[dj][:, f, :, 3 * di + dj, :].unsqueeze(1)\
                            .broadcast_to((W, DIM, 2, HS))
                        prod = stp.tile([W, DIM, SLAB], bf16, tag="prod")
                        pv = prod.rearrange("q c (h r) -> q c h r", h=2)
                        xv = xt_f[:, :, di:di + SLAB]\
                            .rearrange("q c (h r) -> q c h r", h=2)
                        nc.vector.tensor_tensor(pv, xv, kb, Alu.mult)
                        prods.append(prod)
                    a = accs[dj]
                    if f == 0:
                        eng.tensor_tensor(a, prods[0], prods[1], Alu.add)
                    else:
                        eng.tensor_tensor(a, a, prods[0], Alu.add)
                        eng.tensor_tensor(a, a, prods[1], Alu.add)
                    eng.tensor_tensor(a, a, prods[2], Alu.add)

            c_sb = ktp.tile([W, SLAB], f32)
            corr = ktp.tile([W, SLAB], f32)
            c_p1 = ktp.tile([W, SLAB], f32)
            c_m1 = ktp.tile([W, SLAB], f32)
            c_c = ktp.tile([W, SLAB], f32)
            cb_p1 = ktp.tile([W, SLAB], bf16)
            cb_c = ktp.tile([W, SLAB], bf16)
            cb_m1 = ktp.tile([W, SLAB], bf16)
            nc.gpsimd.memset(c_p1[96:128], 0.0)
            nc.gpsimd.memset(c_m1[0:32], 0.0)
            s_sb = accp.tile([W, DIM, SLAB], bf16)

            def emit_S():
                # S = 3-row vertical box of U (edge rows clamped in xt);
                # on gpsimd, in parallel with the last frame's filtering
                nc.gpsimd.tensor_tensor(s_sb, u_sb[:, :, 0:SLAB],
                                        u_sb[:, :, 1:SLAB + 1], Alu.add)
                nc.gpsimd.tensor_tensor(s_sb, s_sb, u_sb[:, :, 2:SLAB + 2],
                                        Alu.add)

            def emit_c_chain():
                # c = 1/45 - mean(ker); sum45 reads the folded kernel, so
                # undo the fold's double-count at the edge partitions.
                nc.vector.tensor_scalar(c_sb, sum45, -1.0 / 45.0, 1.0 / 45.0,
                                        Alu.mult, Alu.add)
                for h in range(2):
                    rh = slice(HS * h, HS * h + HS)
                    ktr_r = kt2[:, :, h, 0:9, :].rearrange(
                        "q f (di dj) r -> q r f di dj", di=3, dj=3)
                    nc.vector.tensor_reduce(corr[0:32, rh],
                                            ktr_r[0:32, :, :, :, 0],
                                            axis=mybir.AxisListType.XY,
                                            op=Alu.add)
                    nc.vector.tensor_reduce(corr[96:128, rh],
                                            ktr_r[96:128, :, :, :, 2],
                                            axis=mybir.AxisListType.XY,
                                            op=Alu.add)
                nc.vector.scalar_tensor_tensor(
                    out=c_sb[0:32], in0=corr[0:32], scalar=ea_sb[0:32],
                    in1=c_sb[0:32], op0=Alu.mult, op1=Alu.add)
                nc.vector.scalar_tensor_tensor(
                    out=c_sb[96:128], in0=corr[96:128], scalar=eb_sb[96:128],
                    in1=c_sb[96:128], op0=Alu.mult, op1=Alu.add)
                # shifted + edge-doubled variants of c, bf16 for 2x filtering
                nc.sync.dma_start(out=c_p1[0:127], in_=c_sb[1:128])
                nc.sync.dma_start(out=c_m1[1:128], in_=c_sb[0:127])
                nc.vector.tensor_scalar(c_c, c_sb, ef_sb, None, Alu.mult)
                nc.gpsimd.tensor_copy(cb_p1, c_p1)
                nc.gpsimd.tensor_copy(cb_c, c_c)
                nc.gpsimd.tensor_copy(cb_m1, c_m1)

            # contiguous per-half staging of the final accumulators (the
            # permute matmul lhsT must have one free dim); written by the
            # cS add, so this costs no extra ops
            accsH = [[accp.tile([W, DIM, HS], bf16, name=f"accH{h}{dj}")
                      for dj in range(3)] for h in range(2)]

            def emit_cs_half(h):
                rh = slice(HS * h, HS * h + HS)
                for dj, csrc in ((0, cb_p1), (1, cb_c), (2, cb_m1)):
                    cbb = csrc[:, rh].unsqueeze(1).broadcast_to((W, DIM, HS))
                    prod = stp.tile([W, DIM, HS], bf16, tag="prod")
                    nc.vector.tensor_tensor(prod, s_sb[:, :, rh], cbb,
                                            Alu.mult)
                    nc.vector.tensor_tensor(accsH[h][dj], accs[dj][:, :, rh],
                                            prod, Alu.add)

            def emit_permute_half(h, psop):
                # fused transpose + dj merge via PSUM-accumulating permute
                # matmuls: out[m,p] = acc1[p,m] + acc0[p-1,m] + acc2[p+1,m]
                rh = slice(HS * h, HS * h + HS)
                af = [a.rearrange("q c r -> q (c r)") for a in accsH[h]]
                for b in range(8):
                    cs = slice(128 * b, 128 * b + 128)
                    l1 = af[1][:, cs]
                    l0 = af[0][:, cs]
                    l2 = af[2][:, cs]
                    pso = psop.tile([128, 128], f32, tag="pso")
                    nc.tensor.matmul(pso, lhsT=l1, rhs=idb_sb,
                                     start=True, stop=False)
                    nc.tensor.matmul(pso, lhsT=l0, rhs=pdn_sb,
                                     start=False, stop=False)
                    nc.tensor.matmul(pso, lhsT=l2, rhs=pup_sb,
                                     start=False, stop=True)
                    ob = obp.tile([128, 128], f32, tag="ob")
                    nc.scalar.activation(ob, pso, Act.Copy, scale=1.0)
                    eng = nc.sync if b % 2 == 0 else nc.scalar
                    eng.dma_start(out=out_d[8 * b:8 * b + 8, rh], in_=ob)

            with (
                tc.tile_pool(name="ps1", bufs=3, space="PSUM") as ps1p,
                tc.tile_pool(name="ps2", bufs=3, space="PSUM") as ps2p,
                tc.tile_pool(name="pso", bufs=2, space="PSUM") as psop,
            ):
                loads = emit_loads(0)
                for f in range(T):
                    nxt = emit_loads(f + 1) if f + 1 < T else None
                    xt_f = loads[2]
                    y2 = emit_conv1(f, loads, ps1p)
                    if f < T - 1:
                        emit_conv2_half(f, 0, y2, ps2p)
                        emit_conv2_half(f, 1, y2, ps2p)
                        emit_filter_full(f, xt_f)
                    else:
                        emit_S()
                        emit_conv2_half(f, 0, y2, ps2p)
                        emit_filter_half(f, 0, xt_f)
                        emit_conv2_half(f, 1, y2, ps2p)
                        if debug:
                            nc.sync.dma_start(out=dbg_kt2, in_=kt2)
                        emit_c_chain()
                        emit_cs_half(0)
                        emit_permute_half(0, psop)
                        emit_filter_half(f, 1, xt_f)
                        emit_cs_half(1)
                        emit_permute_half(1, psop)
                    loads = nxt

    return nc


def _get_program():
    if "nc" not in _PROGRAM_CACHE:
        nc = _build_program()
        nc.finalize()
        _PROGRAM_CACHE["nc"] = nc
    return _PROGRAM_CACHE["nc"]


def _get_program_debug():
    if "ncd" not in _PROGRAM_CACHE:
        nc = _build_program(debug=True)
        nc.finalize()
        _PROGRAM_CACHE["ncd"] = nc
    return _PROGRAM_CACHE["ncd"]


def _host_prep(x, w1, b1, w2, b2):
    """Build the 8 per-core input maps from full inputs."""
    import ml_dtypes
    bf16 = ml_dtypes.bfloat16

    x = np.asarray(x, dtype=np.float32)
    w1 = np.asarray(w1, dtype=np.float32)
    b1 = np.asarray(b1, dtype=np.float32)
    w2 = np.asarray(w2, dtype=np.float32)
    b2 = np.asarray(b2, dtype=np.float32)

    # paired conv weights: [pairs di=0,1 stacked on K, then di=2 single]
    # w1p[ci, dj, o] = w1[o, ci, 0, dj]; w1p[64+ci, dj, o] = w1[o, ci, 1, dj]
    w1p = np.concatenate([w1[:, :, 0, :].transpose(1, 2, 0),
                          w1[:, :, 1, :].transpose(1, 2, 0)], axis=0)
    # w1q pairs taps (2,0)+(2,1) on a column-shifted rhs; w1s2 is tap (2,2)
    w1q = np.concatenate([w1[:, :, 2, 0].T, w1[:, :, 2, 1].T], axis=0)
    w1s2 = np.ascontiguousarray(w1[:, :, 2, 2].T)
    w2p = np.concatenate([w2[:, :, 0, :].transpose(1, 2, 0),
                          w2[:, :, 1, :].transpose(1, 2, 0)], axis=0)
    w2s = np.ascontiguousarray(w2[:, :, 2, :].transpose(1, 2, 0))

    b1c = np.ascontiguousarray(b1.reshape(DIM, 1))
    b2c = np.ascontiguousarray(b2.reshape(9, 1))
    idb = np.eye(128, dtype=np.float32)
    pdn = np.zeros((128, 128), dtype=np.float32)   # pdn[k, p]=1 iff k==p-1
    pdn[np.arange(127), np.arange(1, 128)] = 1.0
    pup = np.zeros((128, 128), dtype=np.float32)   # pup[k, p]=1 iff k==p+1
    pup[np.arange(1, 128), np.arange(127)] = 1.0
    emask = np.zeros((W, 1), dtype=np.float32)
    emask[127, 0] = 1.0
    efold = np.ones((W, 1), dtype=np.float32)
    efold[0, 0] = 2.0
    efold[127, 0] = 2.0
    emA = np.zeros((W, 1), dtype=np.float32)
    emA[0, 0] = 1.0 / 45.0
    emB = np.zeros((W, 1), dtype=np.float32)
    emB[127, 0] = 1.0 / 45.0

    w1p = w1p.astype(bf16)
    w1q = w1q.astype(bf16)
    w1s2 = w1s2.astype(bf16)
    w2p = w2p.astype(bf16)
    w2s = w2s.astype(bf16)
    idb = idb.astype(bf16)
    pdn = pdn.astype(bf16)
    pup = pup.astype(bf16)

    in_maps = []
    for core in range(NCORES):
        b, s = divmod(core, 4)
        r0 = s * SLAB
        # conv input: rows r0-2 .. r0+34 zero padded, cols -1..128 zero padded
        xc = np.zeros((DIM, T, 37, 130), dtype=np.float32)
        lo = max(0, r0 - 2)
        hi = min(H, r0 + 35)
        xc[:, :, lo - (r0 - 2):hi - (r0 - 2), 1:129] = x[b, :, :, lo:hi, :]
        # paired conv rhs tiles: lower half plain, upper half row-shifted
        # (xp) / row-shifted + col-shifted (xq)
        xp = np.concatenate([xc[:, :, 0:36], xc[:, :, 1:37]], axis=0)
        xq_hi = np.zeros((DIM, T, 36, 130), dtype=np.float32)
        xq_hi[:, :, :, 0:129] = xc[:, :, 1:37, 1:130]
        xq = np.concatenate([xc[:, :, 1:37], xq_hi], axis=0)
        # filter input, pixel-partition, innermost rows:
        # xt[q, t, c, j] = x[b, c, t, clip(r0-1+j), q]
        rows = np.clip(np.arange(r0 - 1, r0 + 33), 0, H - 1)
        xt = np.ascontiguousarray(x[b][:, :, rows, :].transpose(3, 1, 0, 2))
        # conv2 zero-pad mask for the y halo rows (y rows 1 and 34)
        ymask = np.ones((128, 2), dtype=np.float32)
        if s == 0:
            ymask[:, 0] = 0.0
        if s == 3:
            ymask[:, 1] = 0.0
        in_maps.append({
            "xp": xp.astype(bf16), "xq": xq.astype(bf16),
            "xt": xt.astype(bf16),
            "w1p": w1p, "w1q": w1q, "w1s2": w1s2, "w2p": w2p, "w2s": w2s,
            "b1c": b1c, "b2c": b2c, "ymask": ymask, "emask": emask,
            "efold": efold, "emA": emA, "emB": emB,
            "idb": idb, "pdn": pdn, "pup": pup,
        })
    return in_maps


def kernel(x, w1, b1, w2, b2):
    from concourse.bass_utils import run_bass_kernel_spmd

    nc = _get_program()
    in_maps = _host_prep(x, w1, b1, w2, b2)
    res = run_bass_kernel_spmd(nc, in_maps, list(range(NCORES)))
    out = np.zeros((2, DIM, H, W), dtype=np.float32)
    for core in range(NCORES):
        b, s = divmod(core, 4)
        out[b, :, s * SLAB:(s + 1) * SLAB, :] = res.results[core]["out"]
    return out


# revision 74
# speedup vs baseline: 1.2734x; 1.2734x over previous
"""Trainium2 Bass kernel for nn_DynamicFiltering.

Computation (per batch b):
  xf = frames of x                     (t, c, h, w)
  y  = LeakyReLU(conv2d(xf, w1, b1), 0.2)
  ker = conv2d(y, w2, b2)              (t, 9, h, w)
  ker = ker - mean_k(ker) + 1/45       (per-pixel kernel over K = t*3*3 = 45)
  out[c,h,w] = sum_{t,k1,k2} x_edge[c,t,h+k1-1,w+k2-1] * ker[t,k1,k2][h,w]

Sharding: 8 cores = 2 batches x 4 H-slabs of 32 rows.

Per-core device program (v2, bf16):
  - conv1/conv2 as bf16 matmuls with K=128 tap pairing: the rhs tiles hold
    the image in partitions 0:64 and a one-row-shifted copy in 64:128, so
    taps (0,dj)+(1,dj) share one matmul and (2,dj) runs on the top half.
    6 matmuls per 4-row chunk instead of 9.
  - LeakyReLU fused into a single scalar-engine Lrelu activation.
  - per-frame kernel transpose to pixel-partition layout via one XBAR DMA
    transpose (16x128 tiles) + one DVE strided copy into [q, tap, r] form.
  - dynamic filtering on DVE in bf16 with every operand innermost-packed
    (xt is [q, c, r], kernel broadcast over c with innermost r) so the
    16-bit 2x mode applies; bf16 accumulators, one per dj column shift.
  - dj merge pre-transpose via DMA partition shifts; 16 f32r PE transposes;
    DMA out.
"""

import numpy as np

DIM = 64
T = 5
H = 128
W = 128
SLAB = 32          # output rows per core
NCORES = 8

_PROGRAM_CACHE = {}


def _build_program(debug=False):
    import concourse.bacc as bacc
    import concourse.mybir as mybir
    from concourse.tile import TileContext

    f32 = mybir.dt.float32
    f32r = mybir.dt.float32r
    bf16 = mybir.dt.bfloat16
    Act = mybir.ActivationFunctionType
    Alu = mybir.AluOpType

    nc = bacc.Bacc("TRN2", debug=False)

    xp_d = nc.dram_tensor("xp", [128, T, 36, 130], bf16, kind="ExternalInput").ap()
    xq_d = nc.dram_tensor("xq", [128, T, 36, 130], bf16, kind="ExternalInput").ap()
    xt_d = nc.dram_tensor("xt", [W, T, DIM, 34], bf16, kind="ExternalInput").ap()
    w1p_d = nc.dram_tensor("w1p", [128, 3, DIM], bf16, kind="ExternalInput").ap()
    w1q_d = nc.dram_tensor("w1q", [128, DIM], bf16, kind="ExternalInput").ap()
    w1s2_d = nc.dram_tensor("w1s2", [64, DIM], bf16, kind="ExternalInput").ap()
    w2p_d = nc.dram_tensor("w2p", [128, 3, 9], bf16, kind="ExternalInput").ap()
    w2s_d = nc.dram_tensor("w2s", [64, 3, 9], bf16, kind="ExternalInput").ap()
    b1_d = nc.dram_tensor("b1c", [DIM, 1], f32, kind="ExternalInput").ap()
    b2_d = nc.dram_tensor("b2c", [9, 1], f32, kind="ExternalInput").ap()
    ym_d = nc.dram_tensor("ymask", [128, 2], f32, kind="ExternalInput").ap()
    em_d = nc.dram_tensor("emask", [W, 1], f32, kind="ExternalInput").ap()
    ef_d = nc.dram_tensor("efold", [W, 1], f32, kind="ExternalInput").ap()
    ea_d = nc.dram_tensor("emA", [W, 1], f32, kind="ExternalInput").ap()
    eb_d = nc.dram_tensor("emB", [W, 1], f32, kind="ExternalInput").ap()
    # permutation matrices for the final fused transpose+shift matmuls
    idb_d = nc.dram_tensor("idb", [128, 128], bf16, kind="ExternalInput").ap()
    pdn_d = nc.dram_tensor("pdn", [128, 128], bf16, kind="ExternalInput").ap()
    pup_d = nc.dram_tensor("pup", [128, 128], bf16, kind="ExternalInput").ap()
    out_d = nc.dram_tensor("out", [DIM, SLAB, W], f32, kind="ExternalOutput").ap()
    if debug:
        dbg_y = nc.dram_tensor("dbg_y", [128, 36, 130], bf16,
                               kind="ExternalOutput").ap()
        dbg_ker = nc.dram_tensor("dbg_ker", [16, SLAB, W], bf16,
                                 kind="ExternalOutput").ap()
        dbg_kta = nc.dram_tensor("dbg_kta", [W, SLAB, 16], bf16,
                                 kind="ExternalOutput").ap()
        dbg_kt2 = nc.dram_tensor("dbg_kt2", [W, T, 2, 16, SLAB // 2], bf16,
                                 kind="ExternalOutput").ap()

    with TileContext(nc) as tc:
        with (
            tc.tile_pool(name="consts", bufs=1) as cpool,
            tc.tile_pool(name="xcp", bufs=2) as xcp,
            tc.tile_pool(name="xtp", bufs=3) as xtp,
            tc.tile_pool(name="yp", bufs=2) as yp,
            tc.tile_pool(name="kerp", bufs=2) as kerp,
            tc.tile_pool(name="ktap", bufs=2) as ktap,
            tc.tile_pool(name="ktp", bufs=1) as ktp,
            tc.tile_pool(name="accp", bufs=1) as accp,
            tc.tile_pool(name="stage", bufs=6) as stp,
            tc.tile_pool(name="obp", bufs=3) as obp,
        ):
            # consts are issued on the scalar-engine DGE so the sync DGE can
            # start streaming frame 0's inputs immediately
            w1p_sb = cpool.tile([128, 3, DIM], bf16)
            nc.scalar.dma_start(out=w1p_sb, in_=w1p_d)
            w1q_sb = cpool.tile([128, DIM], bf16)
            nc.scalar.dma_start(out=w1q_sb, in_=w1q_d)
            w1s2_sb = cpool.tile([64, DIM], bf16)
            nc.scalar.dma_start(out=w1s2_sb, in_=w1s2_d)
            w2p_sb = cpool.tile([128, 3, 9], bf16)
            nc.scalar.dma_start(out=w2p_sb, in_=w2p_d)
            w2s_sb = cpool.tile([128, 3, 9], bf16)
            nc.scalar.dma_start(out=w2s_sb[64:128], in_=w2s_d)
            b1_sb = cpool.tile([DIM, 1], f32)
            nc.scalar.dma_start(out=b1_sb, in_=b1_d)
            b2_sb = cpool.tile([9, 1], f32)
            nc.scalar.dma_start(out=b2_sb, in_=b2_d)
            ym_sb = cpool.tile([128, 2], f32)
            nc.scalar.dma_start(out=ym_sb, in_=ym_d)
            em_sb = cpool.tile([W, 1], f32)
            nc.gpsimd.dma_start(out=em_sb, in_=em_d)
            ef_sb = cpool.tile([W, 1], f32)
            nc.gpsimd.dma_start(out=ef_sb, in_=ef_d)
            ea_sb = cpool.tile([W, 1], f32)
            nc.gpsimd.dma_start(out=ea_sb, in_=ea_d)
            eb_sb = cpool.tile([W, 1], f32)
            nc.gpsimd.dma_start(out=eb_sb, in_=eb_d)
            idb_sb = cpool.tile([128, 128], bf16)
            nc.gpsimd.dma_start(out=idb_sb, in_=idb_d)
            pdn_sb = cpool.tile([128, 128], bf16)
            nc.gpsimd.dma_start(out=pdn_sb, in_=pdn_d)
            pup_sb = cpool.tile([128, 128], bf16)
            nc.gpsimd.dma_start(out=pup_sb, in_=pup_d)
            al_sb = cpool.tile([DIM, 1], f32)
            nc.vector.memset(al_sb, 0.2)

            # per-pixel kernels, [q, frame, r-half, tap16, r16] bf16
            # (taps 9..15 unused; r-half-major so per-half DMAs are
            # contiguous)
            kt2 = ktp.tile([W, T, 2, 16, SLAB // 2], bf16)
            kt_p1 = ktp.tile([W, T, 2, 16, SLAB // 2], bf16)
            kt_m1 = ktp.tile([W, T, 2, 16, SLAB // 2], bf16)
            nc.gpsimd.memset(kt_p1[96:128], 0.0)
            nc.gpsimd.memset(kt_m1[0:32], 0.0)
            sum45 = ktp.tile([W, SLAB], f32)
            t45 = ktp.tile([W, SLAB], f32)

            # bf16 accumulators, one per dj; [q, c, r]
            accs = [accp.tile([W, DIM, SLAB], bf16, name=f"acc{dj}")
                    for dj in range(3)]
            ksrc = [kt_p1, kt2, kt_m1]
            u_sb = accp.tile([W, DIM, 34], bf16)

            def emit_loads(f):
                """Prefetch frame f's inputs; the sync DGE queue carries
                only these bulk loads so they stream FIFO ahead of use."""
                xp = xcp.tile([128, 36, 130], bf16, tag="xp")
                nc.sync.dma_start(out=xp, in_=xp_d[:, f])
                xq = xcp.tile([128, 36, 130], bf16, tag="xq")
                nc.sync.dma_start(out=xq, in_=xq_d[:, f])
                xt_f = xtp.tile([W, DIM, 34], bf16, tag="xt")
                nc.sync.dma_start(out=xt_f, in_=xt_d[:, f])
                return xp, xq, xt_f

            def emit_conv1(f, loads, ps1p):
                xp, xq, xt_f = loads
                y2 = yp.tile([128, 36, 130], bf16, tag="y2")
                nc.gpsimd.memset(y2[:, :, 0:1], 0.0)
                nc.gpsimd.memset(y2[:, :, 129:130], 0.0)
                if f == 0:
                    nc.scalar.activation(u_sb, xt_f, Act.Copy, scale=1.0)
                else:
                    nc.gpsimd.tensor_tensor(u_sb, u_sb, xt_f, Alu.add)

                for rc in range(9):
                    g0 = 1 + 4 * rc
                    nr = 4 if rc < 8 else 2
                    ps = ps1p.tile([DIM, 4, W], f32, tag="ps1")
                    for i, dj in enumerate(range(3)):
                        nc.tensor.matmul(
                            ps[:, :nr, :],
                            lhsT=w1p_sb[:, dj, :],
                            rhs=xp[:, g0 - 1:g0 - 1 + nr, dj:dj + W],
                            start=(i == 0),
                            stop=False,
                        )
                    nc.tensor.matmul(
                        ps[:, :nr, :],
                        lhsT=w1q_sb,
                        rhs=xq[:, g0:g0 + nr, 0:W],
                        start=False,
                        stop=False,
                    )
                    nc.tensor.matmul(
                        ps[:, :nr, :],
                        lhsT=w1s2_sb,
                        rhs=xq[0:64, g0:g0 + nr, 2:2 + W],
                        start=False,
                        stop=True,
                    )
                    nc.scalar.activation(y2[0:64, g0:g0 + nr, 1:129],
                                         ps[:, :nr], Act.Prelu,
                                         bias=b1_sb, scale=1.0, alpha=al_sb)
                    if rc == 0:
                        # conv2 zero-pads rows outside the image: scale the
                        # y rows that fall outside (mask is 0 on edge slabs)
                        nc.scalar.activation(y2[0:64, 1:2, 1:129],
                                             y2[0:64, 1:2, 1:129],
                                             Act.Copy, scale=ym_sb[0:64, 0:1])
                    if rc == 4:
                        # early half of the row-shifted copy for tap pairing
                        nc.gpsimd.dma_start(out=y2[64:128, 0:18],
                                            in_=y2[0:64, 1:19])
                nc.scalar.activation(y2[0:64, 34:35, 1:129],
                                     y2[0:64, 34:35, 1:129],
                                     Act.Copy, scale=ym_sb[0:64, 1:2])
                nc.gpsimd.dma_start(out=y2[64:128, 18:35],
                                    in_=y2[0:64, 19:36])
                if debug and f == 0:
                    nc.sync.dma_start(out=dbg_y, in_=y2)
                return y2

            HS = SLAB // 2  # half-slab rows

            def emit_conv2_half(f, h, y2, ps2p):
                """conv2 for output rows h*16..h*16+16, ker transpose and
                kernel staging for that half."""
                ker16 = kerp.tile([16, HS, W], bf16, tag="ker16")
                for rc4 in range(4):
                    rc = 4 * h + rc4
                    c0 = 2 + 4 * rc
                    ps2 = ps2p.tile([9, 4, W], f32, tag="ps2")
                    for i, dj in enumerate(range(3)):
                        nc.tensor.matmul(
                            ps2,
                            lhsT=w2p_sb[:, dj, :],
                            rhs=y2[:, c0 - 1:c0 + 3, dj:dj + W],
                            start=(i == 0),
                            stop=False,
                        )
                    for i, dj in enumerate(range(3)):
                        nc.tensor.matmul(
                            ps2,
                            lhsT=w2s_sb[64:128, dj, :],
                            rhs=y2[64:128, c0:c0 + 4, dj:dj + W],
                            start=False,
                            stop=(i == 2),
                        )
                    nc.scalar.activation(
                        ker16[0:9, 4 * rc4:4 * rc4 + 4, :],
                        ps2, Act.Identity, bias=b2_sb, scale=1.0)

                # transpose (tap, r, q) -> (q, r, tap) via the DMA XBAR
                # (xbar block b of 128 cols lands at out[:, b, :]),
                # then repack to (q, tap, r) so filtering reads are
                # innermost-contiguous (enables the DVE 16-bit 2x mode)
                rh = slice(HS * h, HS * h + HS)
                kt_a = ktap.tile([W, HS, 16], bf16, tag="kta")
                nc.scalar.dma_start_transpose(
                    out=kt_a, in_=ker16.rearrange("k r q -> k (r q)"))
                nc.vector.tensor_copy(kt2[:, f, h],
                                      kt_a.rearrange("q r t -> q t r"))
                if debug and f == 0:
                    nc.sync.dma_start(out=dbg_ker[:, rh], in_=ker16)
                    nc.sync.dma_start(out=dbg_kta[:, rh], in_=kt_a)

                # fold W-edge replicate-pad terms into the dj=1 slot
                ktr = kt2[:, f, h, 0:9, :].rearrange(
                    "q (di dj) r -> q di dj r", di=3, dj=3)
                nc.vector.tensor_tensor(ktr[0:1, :, 1, :], ktr[0:1, :, 1, :],
                                        ktr[0:1, :, 0, :], Alu.add)
                nc.vector.scalar_tensor_tensor(
                    out=ktr[96:128, :, 1, :],
                    in0=ktr[96:128, :, 2, :], scalar=em_sb[96:128, :],
                    in1=ktr[96:128, :, 1, :], op0=Alu.mult, op1=Alu.add)

                # partition-shifted kernel copies for the dj column shifts
                nc.gpsimd.dma_start(out=kt_p1[0:127, f, h],
                                    in_=kt2[1:128, f, h])
                nc.gpsimd.dma_start(out=kt_m1[1:128, f, h],
                                    in_=kt2[0:127, f, h])

                # incremental sum of the 45 (folded) kernel taps
                t_out = sum45 if f == 0 else t45
                nc.vector.tensor_reduce(
                    t_out[:, rh],
                    kt2[:, f, h, 0:9, :].rearrange("q t r -> q r t"),
                    axis=mybir.AxisListType.X, op=Alu.add)
                if f > 0:
                    nc.vector.tensor_tensor(sum45[:, rh], sum45[:, rh],
                                            t45[:, rh], Alu.add)

            def _acc_engine(dj):
                # dj=2's accumulate chain runs on gpsimd to unload the DVE
                return nc.gpsimd if dj == 2 else nc.vector

            def emit_filter_half(f, h, xt_f):
                rh = slice(HS * h, HS * h + HS)
                for dj in range(3):
                    eng = _acc_engine(dj)
                    prods = []
                    for di in range(3):
                        kb = ksrc[dj][:, f, h, 3 * di + dj, :].unsqueeze(1)\
                            .broadcast_to((W, DIM, HS))
                        prod = stp.tile([W, DIM, HS], bf16, tag="prod")
                        nc.vector.tensor_tensor(
                            prod, xt_f[:, :, HS * h + di:HS * h + di + HS],
                            kb, Alu.mult)
                        prods.append(prod)
                    a = accs[dj][:, :, rh]
                    if f == 0:
                        eng.tensor_tensor(a, prods[0], prods[1], Alu.add)
                    else:
                        eng.tensor_tensor(a, a, prods[0], Alu.add)
                        eng.tensor_tensor(a, a, prods[1], Alu.add)
                    eng.tensor_tensor(a, a, prods[2], Alu.add)

            def emit_filter_full(f, xt_f):
                # full-slab filtering: the kernel operand spans both r-halves
                # via a 4D view (innermost r stays packed for the 2x mode)
                for dj in range(3):
                    eng = _acc_engine(dj)
                    prods = []
                    for di in range(3):
                        kb = ksrc[dj][:, f, :, 3 * di + dj, :].unsqueeze(1)\
                            .broadcast_to((W, DIM, 2, HS))
                        prod = stp.tile([W, DIM, SLAB], bf16, tag="prod")
                        pv = prod.rearrange("q c (h r) -> q c h r", h=2)
                        xv = xt_f[:, :, di:di + SLAB]\
                            .rearrange("q c (h r) -> q c h r", h=2)
                        nc.vector.tensor_tensor(pv, xv, kb, Alu.mult)
                        prods.append(prod)
                    a = accs[dj]
                    if f == 0:
                        eng.tensor_tensor(a, prods[0], prods[1], Alu.add)
                    else:
                        eng.tensor_tensor(a, a, prods[0], Alu.add)
                        eng.tensor_tensor(a, a, prods[1], Alu.add)
                    eng.tensor_tensor(a, a, prods[2], Alu.add)

            c_sb = ktp.tile([W, SLAB], f32)
            corr = ktp.tile([W, SLAB], f32)
            c_p1 = ktp.tile([W, SLAB], f32)
            c_m1 = ktp.tile([W, SLAB], f32)
            c_c = ktp.tile([W, SLAB], f32)
            cb_p1 = ktp.tile([W, SLAB], bf16)
            cb_c = ktp.tile([W, SLAB], bf16)
            cb_m1 = ktp.tile([W, SLAB], bf16)
            nc.gpsimd.memset(c_p1[96:128], 0.0)
            nc.gpsimd.memset(c_m1[0:32], 0.0)
            s_sb = accp.tile([W, DIM, SLAB], bf16)

            def emit_S():
                # S = 3-row vertical box of U (edge rows clamped in xt);
                # on gpsimd, in parallel with the last frame's filtering
                nc.gpsimd.tensor_tensor(s_sb, u_sb[:, :, 0:SLAB],
                                        u_sb[:, :, 1:SLAB + 1], Alu.add)
                nc.gpsimd.tensor_tensor(s_sb, s_sb, u_sb[:, :, 2:SLAB + 2],
                                        Alu.add)

            def emit_c_chain():
                # c = 1/45 - mean(ker); sum45 reads the folded kernel, so
                # undo the fold's double-count at the edge partitions.
                nc.vector.tensor_scalar(c_sb, sum45, -1.0 / 45.0, 1.0 / 45.0,
                                        Alu.mult, Alu.add)
                for h in range(2):
                    rh = slice(HS * h, HS * h + HS)
                    ktr_r = kt2[:, :, h, 0:9, :].rearrange(
                        "q f (di dj) r -> q r f di dj", di=3, dj=3)
                    nc.vector.tensor_reduce(corr[0:32, rh],
                                            ktr_r[0:32, :, :, :, 0],
                                            axis=mybir.AxisListType.XY,
                                            op=Alu.add)
                    nc.vector.tensor_reduce(corr[96:128, rh],
                                            ktr_r[96:128, :, :, :, 2],
                                            axis=mybir.AxisListType.XY,
                                            op=Alu.add)
                nc.vector.scalar_tensor_tensor(
                    out=c_sb[0:32], in0=corr[0:32], scalar=ea_sb[0:32],
                    in1=c_sb[0:32], op0=Alu.mult, op1=Alu.add)
                nc.vector.scalar_tensor_tensor(
                    out=c_sb[96:128], in0=corr[96:128], scalar=eb_sb[96:128],
                    in1=c_sb[96:128], op0=Alu.mult, op1=Alu.add)
                # shifted + edge-doubled variants of c, bf16 for 2x filtering
                nc.sync.dma_start(out=c_p1[0:127], in_=c_sb[1:128])
                nc.sync.dma_start(out=c_m1[1:128], in_=c_sb[0:127])
                nc.vector.tensor_scalar(c_c, c_sb, ef_sb, None, Alu.mult)
                nc.gpsimd.tensor_copy(cb_p1, c_p1)
                nc.gpsimd.tensor_copy(cb_c, c_c)
                nc.gpsimd.tensor_copy(cb_m1, c_m1)

            # contiguous per-half staging of the final accumulators (the
            # permute matmul lhsT must have one free dim); written by the
            # cS add, so this costs no extra ops
            accsH = [[accp.tile([W, DIM, HS], bf16, name=f"accH{h}{dj}")
                      for dj in range(3)] for h in range(2)]

            def emit_cs_half(h):
                rh = slice(HS * h, HS * h + HS)
                for dj, csrc in ((0, cb_p1), (1, cb_c), (2, cb_m1)):
                    cbb = csrc[:, rh].unsqueeze(1).broadcast_to((W, DIM, HS))
                    prod = stp.tile([W, DIM, HS], bf16, tag="prod")
                    nc.vector.tensor_tensor(prod, s_sb[:, :, rh], cbb,
                                            Alu.mult)
                    nc.vector.tensor_tensor(accsH[h][dj], accs[dj][:, :, rh],
                                            prod, Alu.add)

            def emit_permute_half(h, psop):
                # fused transpose + dj merge via PSUM-accumulating permute
                # matmuls: out[m,p] = acc1[p,m] + acc0[p-1,m] + acc2[p+1,m]
                rh = slice(HS * h, HS * h + HS)
                af = [a.rearrange("q c r -> q (c r)") for a in accsH[h]]
                for b in range(8):
                    cs = slice(128 * b, 128 * b + 128)
                    l1 = af[1][:, cs]
                    l0 = af[0][:, cs]
                    l2 = af[2][:, cs]
                    pso = psop.tile([128, 128], f32, tag="pso")
                    nc.tensor.matmul(pso, lhsT=l1, rhs=idb_sb,
                                     start=True, stop=False)
                    nc.tensor.matmul(pso, lhsT=l0, rhs=pdn_sb,
                                     start=False, stop=False)
                    nc.tensor.matmul(pso, lhsT=l2, rhs=pup_sb,
                                     start=False, stop=True)
                    ob = obp.tile([128, 128], f32, tag="ob")
                    nc.scalar.activation(ob, pso, Act.Copy, scale=1.0)
                    eng = nc.sync if b % 2 == 0 else nc.scalar
                    eng.dma_start(out=out_d[8 * b:8 * b + 8, rh], in_=ob)

            with (
                tc.tile_pool(name="ps1", bufs=3, space="PSUM") as ps1p,
                tc.tile_pool(name="ps2", bufs=3, space="PSUM") as ps2p,
                tc.tile_pool(name="pso", bufs=2, space="PSUM") as psop,
            ):
                loads = emit_loads(0)
                for f in range(T):
                    nxt = emit_loads(f + 1) if f + 1 < T else None
                    xt_f = loads[2]
                    y2 = emit_conv1(f, loads, ps1p)
                    if f < T - 1:
                        emit_conv2_half(f, 0, y2, ps2p)
                        emit_filter_half(f, 0, xt_f)
                        emit_conv2_half(f, 1, y2, ps2p)
                        emit_filter_half(f, 1, xt_f)
                    else:
                        emit_S()
                        emit_conv2_half(f, 0, y2, ps2p)
                        emit_filter_half(f, 0, xt_f)
                        emit_conv2_half(f, 1, y2, ps2p)
                        if debug:
                            nc.sync.dma_start(out=dbg_kt2, in_=kt2)
                        emit_c_chain()
                        emit_cs_half(0)
                        emit_permute_half(0, psop)
                        emit_filter_half(f, 1, xt_f)
                        emit_cs_half(1)
                        emit_permute_half(1, psop)
                    loads = nxt

    return nc


def _get_program():
    if "nc" not in _PROGRAM_CACHE:
        nc = _build_program()
        nc.finalize()
        _PROGRAM_CACHE["nc"] = nc
    return _PROGRAM_CACHE["nc"]


def _get_program_debug():
    if "ncd" not in _PROGRAM_CACHE:
        nc = _build_program(debug=True)
        nc.finalize()
        _PROGRAM_CACHE["ncd"] = nc
    return _PROGRAM_CACHE["ncd"]


def _host_prep(x, w1, b1, w2, b2):
    """Build the 8 per-core input maps from full inputs."""
    import ml_dtypes
    bf16 = ml_dtypes.bfloat16

    x = np.asarray(x, dtype=np.float32)
    w1 = np.asarray(w1, dtype=np.float32)
    b1 = np.asarray(b1, dtype=np.float32)
    w2 = np.asarray(w2, dtype=np.float32)
    b2 = np.asarray(b2, dtype=np.float32)

    # paired conv weights: [pairs di=0,1 stacked on K, then di=2 single]
    # w1p[ci, dj, o] = w1[o, ci, 0, dj]; w1p[64+ci, dj, o] = w1[o, ci, 1, dj]
    w1p = np.concatenate([w1[:, :, 0, :].transpose(1, 2, 0),
                          w1[:, :, 1, :].transpose(1, 2, 0)], axis=0)
    # w1q pairs taps (2,0)+(2,1) on a column-shifted rhs; w1s2 is tap (2,2)
    w1q = np.concatenate([w1[:, :, 2, 0].T, w1[:, :, 2, 1].T], axis=0)
    w1s2 = np.ascontiguousarray(w1[:, :, 2, 2].T)
    w2p = np.concatenate([w2[:, :, 0, :].transpose(1, 2, 0),
                          w2[:, :, 1, :].transpose(1, 2, 0)], axis=0)
    w2s = np.ascontiguousarray(w2[:, :, 2, :].transpose(1, 2, 0))

    b1c = np.ascontiguousarray(b1.reshape(DIM, 1))
    b2c = np.ascontiguousarray(b2.reshape(9, 1))
    idb = np.eye(128, dtype=np.float32)
    pdn = np.zeros((128, 128), dtype=np.float32)   # pdn[k, p]=1 iff k==p-1
    pdn[np.arange(127), np.arange(1, 128)] = 1.0
    pup = np.zeros((128, 128), dtype=np.float32)   # pup[k, p]=1 iff k==p+1
    pup[np.arange(1, 128), np.arange(127)] = 1.0
    emask = np.zeros((W, 1), dtype=np.float32)
    emask[127, 0] = 1.0
    efold = np.ones((W, 1), dtype=np.float32)
    efold[0, 0] = 2.0
    efold[127, 0] = 2.0
    emA = np.zeros((W, 1), dtype=np.float32)
    emA[0, 0] = 1.0 / 45.0
    emB = np.zeros((W, 1), dtype=np.float32)
    emB[127, 0] = 1.0 / 45.0

    w1p = w1p.astype(bf16)
    w1q = w1q.astype(bf16)
    w1s2 = w1s2.astype(bf16)
    w2p = w2p.astype(bf16)
    w2s = w2s.astype(bf16)
    idb = idb.astype(bf16)
    pdn = pdn.astype(bf16)
    pup = pup.astype(bf16)

    in_maps = []
    for core in range(NCORES):
        b, s = divmod(core, 4)
        r0 = s * SLAB
        # conv input: rows r0-2 .. r0+34 zero padded, cols -1..128 zero padded
        xc = np.zeros((DIM, T, 37, 130), dtype=np.float32)
        lo = max(0, r0 - 2)
        hi = min(H, r0 + 35)
        xc[:, :, lo - (r0 - 2):hi - (r0 - 2), 1:129] = x[b, :, :, lo:hi, :]
        # paired conv rhs tiles: lower half plain, upper half row-shifted
        # (xp) / row-shifted + col-shifted (xq)
        xp = np.concatenate([xc[:, :, 0:36], xc[:, :, 1:37]], axis=0)
        xq_hi = np.zeros((DIM, T, 36, 130), dtype=np.float32)
        xq_hi[:, :, :, 0:129] = xc[:, :, 1:37, 1:130]
        xq = np.concatenate([xc[:, :, 1:37], xq_hi], axis=0)
        # filter input, pixel-partition, innermost rows:
        # xt[q, t, c, j] = x[b, c, t, clip(r0-1+j), q]
        rows = np.clip(np.arange(r0 - 1, r0 + 33), 0, H - 1)
        xt = np.ascontiguousarray(x[b][:, :, rows, :].transpose(3, 1, 0, 2))
        # conv2 zero-pad mask for the y halo rows (y rows 1 and 34)
        ymask = np.ones((128, 2), dtype=np.float32)
        if s == 0:
            ymask[:, 0] = 0.0
        if s == 3:
            ymask[:, 1] = 0.0
        in_maps.append({
            "xp": xp.astype(bf16), "xq": xq.astype(bf16),
            "xt": xt.astype(bf16),
            "w1p": w1p, "w1q": w1q, "w1s2": w1s2, "w2p": w2p, "w2s": w2s,
            "b1c": b1c, "b2c": b2c, "ymask": ymask, "emask": emask,
            "efold": efold, "emA": emA, "emB": emB,
            "idb": idb, "pdn": pdn, "pup": pup,
        })
    return in_maps


def kernel(x, w1, b1, w2, b2):
    from concourse.bass_utils import run_bass_kernel_spmd

    nc = _get_program()
    in_maps = _host_prep(x, w1, b1, w2, b2)
    res = run_bass_kernel_spmd(nc, in_maps, list(range(NCORES)))
    out = np.zeros((2, DIM, H, W), dtype=np.float32)
    for core in range(NCORES):
        b, s = divmod(core, 4)
        out[b, :, s * SLAB:(s + 1) * SLAB, :] = res.results[core]["out"]
    return out


# revision 77
# speedup vs baseline: 1.3637x; 1.0709x over previous
"""Trainium2 Bass kernel for nn_DynamicFiltering.

Computation (per batch b):
  xf = frames of x                     (t, c, h, w)
  y  = LeakyReLU(conv2d(xf, w1, b1), 0.2)
  ker = conv2d(y, w2, b2)              (t, 9, h, w)
  ker = ker - mean_k(ker) + 1/45       (per-pixel kernel over K = t*3*3 = 45)
  out[c,h,w] = sum_{t,k1,k2} x_edge[c,t,h+k1-1,w+k2-1] * ker[t,k1,k2][h,w]

Sharding: 8 cores = 2 batches x 4 H-slabs of 32 rows.

Per-core device program (bf16):
  - conv1 as 5 and conv2 as 6 bf16 matmuls per 4-row chunk via K=128 tap
    pairing: rhs tiles hold the image in partitions 0:64 and a shifted
    copy in 64:128 (xp: row+1; xq: row+1 / row+1,col+1; y2 gets its
    row-shifted half by an on-chip DMA issued in two early pieces).
  - LeakyReLU fused into one scalar-engine Prelu activation (alpha as a
    per-partition AP; the immediate Lrelu alpha is ignored by HW).
  - per-half-slab kernel transpose to pixel-partition layout via the XBAR
    DMA transpose (out[p,a,b] = in[b, a*128+p]) + a DVE strided repack to
    [q, tap, r] so filtering reads are innermost-contiguous.
  - dynamic filtering on DVE in bf16 with every operand innermost-packed
    (xt is [q, c, r]) so the 16-bit 2x mode applies; bf16 accumulators,
    one per dj column shift, with partition-shifted kernel copies made by
    gpsimd-issued DMAs.
  - the mean-normalization term is decomposed as c*S (c = 1/45 - mean,
    S = box sum of U = sum_t x) and folded into the accumulators.
  - final transpose + dj merge fused into PSUM-accumulating permute
    matmuls (rhs = identity / shifted permutation matrices), so no
    partition-shift DMAs of the big accumulators are needed.
  - input prefetch one frame ahead on a dedicated sync-DGE queue; XBARs
    on the scalar DGE; small latency-critical shifts on the gpsimd DGE.
"""

import numpy as np

DIM = 64
T = 5
H = 128
W = 128
SLAB = 32          # output rows per core
NCORES = 8

_PROGRAM_CACHE = {}


def _build_program(debug=False):
    import concourse.bacc as bacc
    import concourse.mybir as mybir
    from concourse.tile import TileContext

    f32 = mybir.dt.float32
    f32r = mybir.dt.float32r
    bf16 = mybir.dt.bfloat16
    Act = mybir.ActivationFunctionType
    Alu = mybir.AluOpType

    nc = bacc.Bacc("TRN2", debug=False)

    xp_d = nc.dram_tensor("xp", [128, T, 36, 130], bf16, kind="ExternalInput").ap()
    xq_d = nc.dram_tensor("xq", [128, T, 36, 130], bf16, kind="ExternalInput").ap()
    xt_d = nc.dram_tensor("xt", [W, T, DIM, 34], bf16, kind="ExternalInput").ap()
    w1p_d = nc.dram_tensor("w1p", [128, 3, DIM], bf16, kind="ExternalInput").ap()
    w1q_d = nc.dram_tensor("w1q", [128, DIM], bf16, kind="ExternalInput").ap()
    w1s2_d = nc.dram_tensor("w1s2", [64, DIM], bf16, kind="ExternalInput").ap()
    w2p_d = nc.dram_tensor("w2p", [128, 3, 9], bf16, kind="ExternalInput").ap()
    w2s_d = nc.dram_tensor("w2s", [64, 3, 9], bf16, kind="ExternalInput").ap()
    b1_d = nc.dram_tensor("b1c", [DIM, 1], f32, kind="ExternalInput").ap()
    b2_d = nc.dram_tensor("b2c", [9, 1], f32, kind="ExternalInput").ap()
    ym_d = nc.dram_tensor("ymask", [128, 2], f32, kind="ExternalInput").ap()
    em_d = nc.dram_tensor("emask", [W, 1], f32, kind="ExternalInput").ap()
    ef_d = nc.dram_tensor("efold", [W, 1], f32, kind="ExternalInput").ap()
    ea_d = nc.dram_tensor("emA", [W, 1], f32, kind="ExternalInput").ap()
    eb_d = nc.dram_tensor("emB", [W, 1], f32, kind="ExternalInput").ap()
    # permutation matrices for the final fused transpose+shift matmuls
    idb_d = nc.dram_tensor("idb", [128, 128], bf16, kind="ExternalInput").ap()
    pdn_d = nc.dram_tensor("pdn", [128, 128], bf16, kind="ExternalInput").ap()
    pup_d = nc.dram_tensor("pup", [128, 128], bf16, kind="ExternalInput").ap()
    out_d = nc.dram_tensor("out", [DIM, SLAB, W], f32, kind="ExternalOutput").ap()
    if debug:
        dbg_y = nc.dram_tensor("dbg_y", [128, 36, 130], bf16,
                               kind="ExternalOutput").ap()
        dbg_ker = nc.dram_tensor("dbg_ker", [16, SLAB, W], bf16,
                                 kind="ExternalOutput").ap()
        dbg_kta = nc.dram_tensor("dbg_kta", [W, SLAB, 16], bf16,
                                 kind="ExternalOutput").ap()
        dbg_kt2 = nc.dram_tensor("dbg_kt2", [W, T, 2, 16, SLAB // 2], bf16,
                                 kind="ExternalOutput").ap()

    with TileContext(nc) as tc:
        with (
            tc.tile_pool(name="consts", bufs=1) as cpool,
            tc.tile_pool(name="xcp", bufs=2) as xcp,
            tc.tile_pool(name="xtp", bufs=3) as xtp,
            tc.tile_pool(name="yp", bufs=2) as yp,
            tc.tile_pool(name="kerp", bufs=2) as kerp,
            tc.tile_pool(name="ktap", bufs=2) as ktap,
            tc.tile_pool(name="ktp", bufs=1) as ktp,
            tc.tile_pool(name="accp", bufs=1) as accp,
            tc.tile_pool(name="stage", bufs=6) as stp,
            tc.tile_pool(name="obp", bufs=3) as obp,
        ):
            # consts are issued on the scalar-engine DGE so the sync DGE can
            # start streaming frame 0's inputs immediately
            w1p_sb = cpool.tile([128, 3, DIM], bf16)
            nc.scalar.dma_start(out=w1p_sb, in_=w1p_d)
            w1q_sb = cpool.tile([128, DIM], bf16)
            nc.scalar.dma_start(out=w1q_sb, in_=w1q_d)
            w1s2_sb = cpool.tile([64, DIM], bf16)
            nc.scalar.dma_start(out=w1s2_sb, in_=w1s2_d)
            w2p_sb = cpool.tile([128, 3, 9], bf16)
            nc.scalar.dma_start(out=w2p_sb, in_=w2p_d)
            w2s_sb = cpool.tile([128, 3, 9], bf16)
            nc.scalar.dma_start(out=w2s_sb[64:128], in_=w2s_d)
            b1_sb = cpool.tile([DIM, 1], f32)
            nc.scalar.dma_start(out=b1_sb, in_=b1_d)
            b2_sb = cpool.tile([9, 1], f32)
            nc.scalar.dma_start(out=b2_sb, in_=b2_d)
            ym_sb = cpool.tile([128, 2], f32)
            nc.scalar.dma_start(out=ym_sb, in_=ym_d)
            em_sb = cpool.tile([W, 1], f32)
            nc.gpsimd.dma_start(out=em_sb, in_=em_d)
            ef_sb = cpool.tile([W, 1], f32)
            nc.gpsimd.dma_start(out=ef_sb, in_=ef_d)
            ea_sb = cpool.tile([W, 1], f32)
            nc.gpsimd.dma_start(out=ea_sb, in_=ea_d)
            eb_sb = cpool.tile([W, 1], f32)
            nc.gpsimd.dma_start(out=eb_sb, in_=eb_d)
            idb_sb = cpool.tile([128, 128], bf16)
            nc.gpsimd.dma_start(out=idb_sb, in_=idb_d)
            pdn_sb = cpool.tile([128, 128], bf16)
            nc.gpsimd.dma_start(out=pdn_sb, in_=pdn_d)
            pup_sb = cpool.tile([128, 128], bf16)
            nc.gpsimd.dma_start(out=pup_sb, in_=pup_d)
            al_sb = cpool.tile([DIM, 1], f32)
            nc.vector.memset(al_sb, 0.2)

            # per-pixel kernels, [q, frame, r-half, tap16, r16] bf16
            # (taps 9..15 unused; r-half-major so per-half DMAs are
            # contiguous)
            kt2 = ktp.tile([W, T, 2, 16, SLAB // 2], bf16)
            kt_p1 = ktp.tile([W, T, 2, 16, SLAB // 2], bf16)
            kt_m1 = ktp.tile([W, T, 2, 16, SLAB // 2], bf16)
            nc.gpsimd.memset(kt_p1[96:128], 0.0)
            nc.gpsimd.memset(kt_m1[0:32], 0.0)
            sum45 = ktp.tile([W, SLAB], f32)
            t45 = ktp.tile([W, SLAB], f32)

            # bf16 accumulators, one per dj; [q, c, r]
            accs = [accp.tile([W, DIM, SLAB], bf16, name=f"acc{dj}")
                    for dj in range(3)]
            ksrc = [kt_p1, kt2, kt_m1]
            u_sb = accp.tile([W, DIM, 34], bf16)

            def emit_loads(f):
                """Prefetch frame f's inputs; the sync DGE queue carries
                only these bulk loads so they stream FIFO ahead of use."""
                xp = xcp.tile([128, 36, 130], bf16, tag="xp")
                nc.sync.dma_start(out=xp, in_=xp_d[:, f])
                xq = xcp.tile([128, 36, 130], bf16, tag="xq")
                nc.sync.dma_start(out=xq, in_=xq_d[:, f])
                xt_f = xtp.tile([W, DIM, 34], bf16, tag="xt")
                nc.sync.dma_start(out=xt_f, in_=xt_d[:, f])
                return xp, xq, xt_f

            def emit_conv1(f, loads, ps1p):
                xp, xq, xt_f = loads
                y2 = yp.tile([128, 36, 130], bf16, tag="y2")
                nc.gpsimd.memset(y2[:, :, 0:1], 0.0)
                nc.gpsimd.memset(y2[:, :, 129:130], 0.0)
                if f == 0:
                    nc.scalar.activation(u_sb, xt_f, Act.Copy, scale=1.0)
                else:
                    nc.gpsimd.tensor_tensor(u_sb, u_sb, xt_f, Alu.add)

                for rc in range(9):
                    g0 = 1 + 4 * rc
                    nr = 4 if rc < 8 else 2
                    ps = ps1p.tile([DIM, 4, W], f32, tag="ps1")
                    for i, dj in enumerate(range(3)):
                        nc.tensor.matmul(
                            ps[:, :nr, :],
                            lhsT=w1p_sb[:, dj, :],
                            rhs=xp[:, g0 - 1:g0 - 1 + nr, dj:dj + W],
                            start=(i == 0),
                            stop=False,
                        )
                    nc.tensor.matmul(
                        ps[:, :nr, :],
                        lhsT=w1q_sb,
                        rhs=xq[:, g0:g0 + nr, 0:W],
                        start=False,
                        stop=False,
                    )
                    nc.tensor.matmul(
                        ps[:, :nr, :],
                        lhsT=w1s2_sb,
                        rhs=xq[0:64, g0:g0 + nr, 2:2 + W],
                        start=False,
                        stop=True,
                    )
                    nc.scalar.activation(y2[0:64, g0:g0 + nr, 1:129],
                                         ps[:, :nr], Act.Prelu,
                                         bias=b1_sb, scale=1.0, alpha=al_sb)
                    if rc == 0:
                        # conv2 zero-pads rows outside the image: scale the
                        # y rows that fall outside (mask is 0 on edge slabs)
                        nc.scalar.activation(y2[0:64, 1:2, 1:129],
                                             y2[0:64, 1:2, 1:129],
                                             Act.Copy, scale=ym_sb[0:64, 0:1])
                    if rc == 4:
                        # early half of the row-shifted copy for tap pairing
                        nc.gpsimd.dma_start(out=y2[64:128, 0:18],
                                            in_=y2[0:64, 1:19])
                nc.scalar.activation(y2[0:64, 34:35, 1:129],
                                     y2[0:64, 34:35, 1:129],
                                     Act.Copy, scale=ym_sb[0:64, 1:2])
                nc.gpsimd.dma_start(out=y2[64:128, 18:35],
                                    in_=y2[0:64, 19:36])
                if debug and f == 0:
                    nc.sync.dma_start(out=dbg_y, in_=y2)
                return y2

            HS = SLAB // 2  # half-slab rows

            def emit_conv2_half(f, h, y2, ps2p):
                """conv2 for output rows h*16..h*16+16, ker transpose and
                kernel staging for that half."""
                ker16 = kerp.tile([16, HS, W], bf16, tag="ker16")
                for rc4 in range(4):
                    rc = 4 * h + rc4
                    c0 = 2 + 4 * rc
                    ps2 = ps2p.tile([9, 4, W], f32, tag="ps2")
                    for i, dj in enumerate(range(3)):
                        nc.tensor.matmul(
                            ps2,
                            lhsT=w2p_sb[:, dj, :],
                            rhs=y2[:, c0 - 1:c0 + 3, dj:dj + W],
                            start=(i == 0),
                            stop=False,
                        )
                    for i, dj in enumerate(range(3)):
                        nc.tensor.matmul(
                            ps2,
                            lhsT=w2s_sb[64:128, dj, :],
                            rhs=y2[64:128, c0:c0 + 4, dj:dj + W],
                            start=False,
                            stop=(i == 2),
                        )
                    nc.scalar.activation(
                        ker16[0:9, 4 * rc4:4 * rc4 + 4, :],
                        ps2, Act.Identity, bias=b2_sb, scale=1.0)

                # transpose (tap, r, q) -> (q, r, tap) via the DMA XBAR
                # (xbar block b of 128 cols lands at out[:, b, :]),
                # then repack to (q, tap, r) so filtering reads are
                # innermost-contiguous (enables the DVE 16-bit 2x mode)
                rh = slice(HS * h, HS * h + HS)
                kt_a = ktap.tile([W, HS, 16], bf16, tag="kta")
                nc.scalar.dma_start_transpose(
                    out=kt_a, in_=ker16.rearrange("k r q -> k (r q)"))
                nc.vector.tensor_copy(kt2[:, f, h],
                                      kt_a.rearrange("q r t -> q t r"))
                if debug and f == 0:
                    nc.sync.dma_start(out=dbg_ker[:, rh], in_=ker16)
                    nc.sync.dma_start(out=dbg_kta[:, rh], in_=kt_a)

                # fold W-edge replicate-pad terms into the dj=1 slot
                ktr = kt2[:, f, h, 0:9, :].rearrange(
                    "q (di dj) r -> q di dj r", di=3, dj=3)
                nc.vector.tensor_tensor(ktr[0:1, :, 1, :], ktr[0:1, :, 1, :],
                                        ktr[0:1, :, 0, :], Alu.add)
                nc.vector.scalar_tensor_tensor(
                    out=ktr[96:128, :, 1, :],
                    in0=ktr[96:128, :, 2, :], scalar=em_sb[96:128, :],
                    in1=ktr[96:128, :, 1, :], op0=Alu.mult, op1=Alu.add)

                # partition-shifted kernel copies for the dj column shifts
                nc.gpsimd.dma_start(out=kt_p1[0:127, f, h],
                                    in_=kt2[1:128, f, h])
                nc.gpsimd.dma_start(out=kt_m1[1:128, f, h],
                                    in_=kt2[0:127, f, h])

                # incremental sum of the 45 (folded) kernel taps
                t_out = sum45 if f == 0 else t45
                nc.vector.tensor_reduce(
                    t_out[:, rh],
                    kt2[:, f, h, 0:9, :].rearrange("q t r -> q r t"),
                    axis=mybir.AxisListType.X, op=Alu.add)
                if f > 0:
                    nc.vector.tensor_tensor(sum45[:, rh], sum45[:, rh],
                                            t45[:, rh], Alu.add)

            def _acc_engine(dj):
                return nc.vector

            def emit_filter_half(f, h, xt_f):
                rh = slice(HS * h, HS * h + HS)
                for dj in range(3):
                    eng = _acc_engine(dj)
                    prods = []
                    for di in range(3):
                        kb = ksrc[dj][:, f, h, 3 * di + dj, :].unsqueeze(1)\
                            .broadcast_to((W, DIM, HS))
                        prod = stp.tile([W, DIM, HS], bf16, tag="prod")
                        nc.vector.tensor_tensor(
                            prod, xt_f[:, :, HS * h + di:HS * h + di + HS],
                            kb, Alu.mult)
                        prods.append(prod)
                    a = accs[dj][:, :, rh]
                    if f == 0:
                        eng.tensor_tensor(a, prods[0], prods[1], Alu.add)
                    else:
                        eng.tensor_tensor(a, a, prods[0], Alu.add)
                        eng.tensor_tensor(a, a, prods[1], Alu.add)
                    eng.tensor_tensor(a, a, prods[2], Alu.add)

            def emit_filter_full(f, xt_f):
                # full-slab filtering: the kernel operand spans both r-halves
                # via a 4D view (innermost r stays packed for the 2x mode)
                for dj in range(3):
                    eng = _acc_engine(dj)
                    prods = []
                    for di in range(3):
                        kb = ksrc[dj][:, f, :, 3 * di + dj, :].unsqueeze(1)\
                            .broadcast_to((W, DIM, 2, HS))
                        prod = stp.tile([W, DIM, SLAB], bf16, tag="prod")
                        pv = prod.rearrange("q c (h r) -> q c h r", h=2)
                        xv = xt_f[:, :, di:di + SLAB]\
                            .rearrange("q c (h r) -> q c h r", h=2)
                        nc.vector.tensor_tensor(pv, xv, kb, Alu.mult)
                        prods.append(prod)
                    a = accs[dj]
                    if f == 0:
                        eng.tensor_tensor(a, prods[0], prods[1], Alu.add)
                    else:
                        eng.tensor_tensor(a, a, prods[0], Alu.add)
                        eng.tensor_tensor(a, a, prods[1], Alu.add)
                    eng.tensor_tensor(a, a, prods[2], Alu.add)

            c_sb = ktp.tile([W, SLAB], f32)
            corr = ktp.tile([W, SLAB], f32)
            c_p1 = ktp.tile([W, SLAB], f32)
            c_m1 = ktp.tile([W, SLAB], f32)
            c_c = ktp.tile([W, SLAB], f32)
            cb_p1 = ktp.tile([W, SLAB], bf16)
            cb_c = ktp.tile([W, SLAB], bf16)
            cb_m1 = ktp.tile([W, SLAB], bf16)
            nc.gpsimd.memset(c_p1[96:128], 0.0)
            nc.gpsimd.memset(c_m1[0:32], 0.0)
            s_sb = accp.tile([W, DIM, SLAB], bf16)

            def emit_S():
                # S = 3-row vertical box of U (edge rows clamped in xt);
                # on gpsimd, in parallel with the last frame's filtering
                nc.gpsimd.tensor_tensor(s_sb, u_sb[:, :, 0:SLAB],
                                        u_sb[:, :, 1:SLAB + 1], Alu.add)
                nc.gpsimd.tensor_tensor(s_sb, s_sb, u_sb[:, :, 2:SLAB + 2],
                                        Alu.add)

            def emit_c_chain():
                # c = 1/45 - mean(ker); sum45 reads the folded kernel, so
                # undo the fold's double-count at the edge partitions.
                nc.vector.tensor_scalar(c_sb, sum45, -1.0 / 45.0, 1.0 / 45.0,
                                        Alu.mult, Alu.add)
                for h in range(2):
                    rh = slice(HS * h, HS * h + HS)
                    ktr_r = kt2[:, :, h, 0:9, :].rearrange(
                        "q f (di dj) r -> q r f di dj", di=3, dj=3)
                    nc.vector.tensor_reduce(corr[0:32, rh],
                                            ktr_r[0:32, :, :, :, 0],
                                            axis=mybir.AxisListType.XY,
                                            op=Alu.add)
                    nc.vector.tensor_reduce(corr[96:128, rh],
                                            ktr_r[96:128, :, :, :, 2],
                                            axis=mybir.AxisListType.XY,
                                            op=Alu.add)
                nc.vector.scalar_tensor_tensor(
                    out=c_sb[0:32], in0=corr[0:32], scalar=ea_sb[0:32],
                    in1=c_sb[0:32], op0=Alu.mult, op1=Alu.add)
                nc.vector.scalar_tensor_tensor(
                    out=c_sb[96:128], in0=corr[96:128], scalar=eb_sb[96:128],
                    in1=c_sb[96:128], op0=Alu.mult, op1=Alu.add)
                # shifted + edge-doubled variants of c, bf16 for 2x filtering
                nc.sync.dma_start(out=c_p1[0:127], in_=c_sb[1:128])
                nc.sync.dma_start(out=c_m1[1:128], in_=c_sb[0:127])
                nc.vector.tensor_scalar(c_c, c_sb, ef_sb, None, Alu.mult)
                nc.gpsimd.tensor_copy(cb_p1, c_p1)
                nc.gpsimd.tensor_copy(cb_c, c_c)
                nc.gpsimd.tensor_copy(cb_m1, c_m1)

            # contiguous per-half staging of the final accumulators (the
            # permute matmul lhsT must have one free dim); written by the
            # cS add, so this costs no extra ops
            accsH = [[accp.tile([W, DIM, HS], bf16, name=f"accH{h}{dj}")
                      for dj in range(3)] for h in range(2)]

            def emit_cs_half(h):
                rh = slice(HS * h, HS * h + HS)
                for dj, csrc in ((0, cb_p1), (1, cb_c), (2, cb_m1)):
                    cbb = csrc[:, rh].unsqueeze(1).broadcast_to((W, DIM, HS))
                    prod = stp.tile([W, DIM, HS], bf16, tag="prod")
                    nc.vector.tensor_tensor(prod, s_sb[:, :, rh], cbb,
                                            Alu.mult)
                    nc.vector.tensor_tensor(accsH[h][dj], accs[dj][:, :, rh],
                                            prod, Alu.add)

            def emit_permute_half(h, psop):
                # fused transpose + dj merge via PSUM-accumulating permute
                # matmuls: out[m,p] = acc1[p,m] + acc0[p-1,m] + acc2[p+1,m]
                rh = slice(HS * h, HS * h + HS)
                af = [a.rearrange("q c r -> q (c r)") for a in accsH[h]]
                for b in range(8):
                    cs = slice(128 * b, 128 * b + 128)
                    l1 = af[1][:, cs]
                    l0 = af[0][:, cs]
                    l2 = af[2][:, cs]
                    pso = psop.tile([128, 128], f32, tag="pso")
                    nc.tensor.matmul(pso, lhsT=l1, rhs=idb_sb,
                                     start=True, stop=False)
                    nc.tensor.matmul(pso, lhsT=l0, rhs=pdn_sb,
                                     start=False, stop=False)
                    nc.tensor.matmul(pso, lhsT=l2, rhs=pup_sb,
                                     start=False, stop=True)
                    ob = obp.tile([128, 128], f32, tag="ob")
                    nc.scalar.activation(ob, pso, Act.Copy, scale=1.0)
                    eng = nc.sync if b % 2 == 0 else nc.scalar
                    eng.dma_start(out=out_d[8 * b:8 * b + 8, rh], in_=ob)

            with (
                tc.tile_pool(name="ps1", bufs=3, space="PSUM") as ps1p,
                tc.tile_pool(name="ps2", bufs=3, space="PSUM") as ps2p,
                tc.tile_pool(name="pso", bufs=2, space="PSUM") as psop,
            ):
                loads = emit_loads(0)
                for f in range(T):
                    nxt = emit_loads(f + 1) if f + 1 < T else None
                    xt_f = loads[2]
                    y2 = emit_conv1(f, loads, ps1p)
                    if f < T - 1:
                        emit_conv2_half(f, 0, y2, ps2p)
                        emit_conv2_half(f, 1, y2, ps2p)
                        emit_filter_full(f, xt_f)
                    else:
                        emit_S()
                        emit_conv2_half(f, 0, y2, ps2p)
                        emit_filter_half(f, 0, xt_f)
                        emit_conv2_half(f, 1, y2, ps2p)
                        if debug:
                            nc.sync.dma_start(out=dbg_kt2, in_=kt2)
                        emit_c_chain()
                        emit_cs_half(0)
                        emit_permute_half(0, psop)
                        emit_filter_half(f, 1, xt_f)
                        emit_cs_half(1)
                        emit_permute_half(1, psop)
                    loads = nxt

    return nc


def _get_program():
    if "nc" not in _PROGRAM_CACHE:
        nc = _build_program()
        nc.finalize()
        _PROGRAM_CACHE["nc"] = nc
    return _PROGRAM_CACHE["nc"]


def _get_program_debug():
    if "ncd" not in _PROGRAM_CACHE:
        nc = _build_program(debug=True)
        nc.finalize()
        _PROGRAM_CACHE["ncd"] = nc
    return _PROGRAM_CACHE["ncd"]


def _host_prep(x, w1, b1, w2, b2):
    """Build the 8 per-core input maps from full inputs."""
    import ml_dtypes
    bf16 = ml_dtypes.bfloat16

    x = np.asarray(x, dtype=np.float32)
    w1 = np.asarray(w1, dtype=np.float32)
    b1 = np.asarray(b1, dtype=np.float32)
    w2 = np.asarray(w2, dtype=np.float32)
    b2 = np.asarray(b2, dtype=np.float32)

    # paired conv weights: [pairs di=0,1 stacked on K, then di=2 single]
    # w1p[ci, dj, o] = w1[o, ci, 0, dj]; w1p[64+ci, dj, o] = w1[o, ci, 1, dj]
    w1p = np.concatenate([w1[:, :, 0, :].transpose(1, 2, 0),
                          w1[:, :, 1, :].transpose(1, 2, 0)], axis=0)
    # w1q pairs taps (2,0)+(2,1) on a column-shifted rhs; w1s2 is tap (2,2)
    w1q = np.concatenate([w1[:, :, 2, 0].T, w1[:, :, 2, 1].T], axis=0)
    w1s2 = np.ascontiguousarray(w1[:, :, 2, 2].T)
    w2p = np.concatenate([w2[:, :, 0, :].transpose(1, 2, 0),
                          w2[:, :, 1, :].transpose(1, 2, 0)], axis=0)
    w2s = np.ascontiguousarray(w2[:, :, 2, :].transpose(1, 2, 0))

    b1c = np.ascontiguousarray(b1.reshape(DIM, 1))
    b2c = np.ascontiguousarray(b2.reshape(9, 1))
    idb = np.eye(128, dtype=np.float32)
    pdn = np.zeros((128, 128), dtype=np.float32)   # pdn[k, p]=1 iff k==p-1
    pdn[np.arange(127), np.arange(1, 128)] = 1.0
    pup = np.zeros((128, 128), dtype=np.float32)   # pup[k, p]=1 iff k==p+1
    pup[np.arange(1, 128), np.arange(127)] = 1.0
    emask = np.zeros((W, 1), dtype=np.float32)
    emask[127, 0] = 1.0
    efold = np.ones((W, 1), dtype=np.float32)
    efold[0, 0] = 2.0
    efold[127, 0] = 2.0
    emA = np.zeros((W, 1), dtype=np.float32)
    emA[0, 0] = 1.0 / 45.0
    emB = np.zeros((W, 1), dtype=np.float32)
    emB[127, 0] = 1.0 / 45.0

    w1p = w1p.astype(bf16)
    w1q = w1q.astype(bf16)
    w1s2 = w1s2.astype(bf16)
    w2p = w2p.astype(bf16)
    w2s = w2s.astype(bf16)
    idb = idb.astype(bf16)
    pdn = pdn.astype(bf16)
    pup = pup.astype(bf16)

    in_maps = []
    for core in range(NCORES):
        b, s = divmod(core, 4)
        r0 = s * SLAB
        # conv input: rows r0-2 .. r0+34 zero padded, cols -1..128 zero padded
        xc = np.zeros((DIM, T, 37, 130), dtype=np.float32)
        lo = max(0, r0 - 2)
        hi = min(H, r0 + 35)
        xc[:, :, lo - (r0 - 2):hi - (r0 - 2), 1:129] = x[b, :, :, lo:hi, :]
        # paired conv rhs tiles: lower half plain, upper half row-shifted
        # (xp) / row-shifted + col-shifted (xq)
        xp = np.concatenate([xc[:, :, 0:36], xc[:, :, 1:37]], axis=0)
        xq_hi = np.zeros((DIM, T, 36, 130), dtype=np.float32)
        xq_hi[:, :, :, 0:129] = xc[:, :, 1:37, 1:130]
        xq = np.concatenate([xc[:, :, 1:37], xq_hi], axis=0)
        # filter input, pixel-partition, innermost rows:
        # xt[q, t, c, j] = x[b, c, t, clip(r0-1+j), q]
        rows = np.clip(np.arange(r0 - 1, r0 + 33), 0, H - 1)
        xt = np.ascontiguousarray(x[b][:, :, rows, :].transpose(3, 1, 0, 2))
        # conv2 zero-pad mask for the y halo rows (y rows 1 and 34)
        ymask = np.ones((128, 2), dtype=np.float32)
        if s == 0:
            ymask[:, 0] = 0.0
        if s == 3:
            ymask[:, 1] = 0.0
        in_maps.append({
            "xp": xp.astype(bf16), "xq": xq.astype(bf16),
            "xt": xt.astype(bf16),
            "w1p": w1p, "w1q": w1q, "w1s2": w1s2, "w2p": w2p, "w2s": w2s,
            "b1c": b1c, "b2c": b2c, "ymask": ymask, "emask": emask,
            "efold": efold, "emA": emA, "emB": emB,
            "idb": idb, "pdn": pdn, "pup": pup,
        })
    return in_maps


def kernel(x, w1, b1, w2, b2):
    from concourse.bass_utils import run_bass_kernel_spmd

    nc = _get_program()
    in_maps = _host_prep(x, w1, b1, w2, b2)
    res = run_bass_kernel_spmd(nc, in_maps, list(range(NCORES)))
    out = np.zeros((2, DIM, H, W), dtype=np.float32)
    for core in range(NCORES):
        b, s = divmod(core, 4)
        out[b, :, s * SLAB:(s + 1) * SLAB, :] = res.results[core]["out"]
    return out


# revision 78
# speedup vs baseline: 1.4320x; 1.0501x over previous
"""Trainium2 Bass kernel for nn_DynamicFiltering.

Computation (per batch b):
  xf = frames of x                     (t, c, h, w)
  y  = LeakyReLU(conv2d(xf, w1, b1), 0.2)
  ker = conv2d(y, w2, b2)              (t, 9, h, w)
  ker = ker - mean_k(ker) + 1/45       (per-pixel kernel over K = t*3*3 = 45)
  out[c,h,w] = sum_{t,k1,k2} x_edge[c,t,h+k1-1,w+k2-1] * ker[t,k1,k2][h,w]

Sharding: 8 cores = 2 batches x 4 H-slabs of 32 rows.

Per-core device program (bf16):
  - conv1 as 5 and conv2 as 6 bf16 matmuls per 4-row chunk via K=128 tap
    pairing: rhs tiles hold the image in partitions 0:64 and a shifted
    copy in 64:128 (xp: row+1; xq: row+1 / row+1,col+1; y2 gets its
    row-shifted half by an on-chip DMA issued in two early pieces).
  - LeakyReLU fused into one scalar-engine Prelu activation (alpha as a
    per-partition AP; the immediate Lrelu alpha is ignored by HW).
  - per-half-slab kernel transpose to pixel-partition layout via the XBAR
    DMA transpose (out[p,a,b] = in[b, a*128+p]) + a DVE strided repack to
    [q, tap, r] so filtering reads are innermost-contiguous.
  - dynamic filtering on DVE in bf16 with every operand innermost-packed
    (xt is [q, c, r]) so the 16-bit 2x mode applies; bf16 accumulators,
    one per dj column shift, with partition-shifted kernel copies made by
    gpsimd-issued DMAs.
  - the mean-normalization term is decomposed as c*S (c = 1/45 - mean,
    S = box sum of U = sum_t x) and folded into the accumulators.
  - final transpose + dj merge fused into PSUM-accumulating permute
    matmuls (rhs = identity / shifted permutation matrices), so no
    partition-shift DMAs of the big accumulators are needed.
  - input prefetch one frame ahead on a dedicated sync-DGE queue; XBARs
    on the scalar DGE; small latency-critical shifts on the gpsimd DGE.
"""

import numpy as np

DIM = 64
T = 5
H = 128
W = 128
SLAB = 32          # output rows per core
NCORES = 8

_PROGRAM_CACHE = {}


def _build_program(debug=False):
    import concourse.bacc as bacc
    import concourse.mybir as mybir
    from concourse.tile import TileContext

    f32 = mybir.dt.float32
    f32r = mybir.dt.float32r
    bf16 = mybir.dt.bfloat16
    Act = mybir.ActivationFunctionType
    Alu = mybir.AluOpType

    nc = bacc.Bacc("TRN2", debug=False)

    xp_d = nc.dram_tensor("xp", [128, T, 36, 130], bf16, kind="ExternalInput").ap()
    xq_d = nc.dram_tensor("xq", [128, T, 36, 130], bf16, kind="ExternalInput").ap()
    xt_d = nc.dram_tensor("xt", [W, T, DIM, 34], bf16, kind="ExternalInput").ap()
    w1p_d = nc.dram_tensor("w1p", [128, 3, DIM], bf16, kind="ExternalInput").ap()
    w1q_d = nc.dram_tensor("w1q", [128, DIM], bf16, kind="ExternalInput").ap()
    w1s2_d = nc.dram_tensor("w1s2", [64, DIM], bf16, kind="ExternalInput").ap()
    w2p_d = nc.dram_tensor("w2p", [128, 3, 9], bf16, kind="ExternalInput").ap()
    w2s_d = nc.dram_tensor("w2s", [64, 3, 9], bf16, kind="ExternalInput").ap()
    b1_d = nc.dram_tensor("b1c", [DIM, 1], f32, kind="ExternalInput").ap()
    b2_d = nc.dram_tensor("b2c", [9, 1], f32, kind="ExternalInput").ap()
    ym_d = nc.dram_tensor("ymask", [128, 2], f32, kind="ExternalInput").ap()
    em_d = nc.dram_tensor("emask", [W, 1], f32, kind="ExternalInput").ap()
    ef_d = nc.dram_tensor("efold", [W, 1], f32, kind="ExternalInput").ap()
    ea_d = nc.dram_tensor("emA", [W, 1], f32, kind="ExternalInput").ap()
    eb_d = nc.dram_tensor("emB", [W, 1], f32, kind="ExternalInput").ap()
    # permutation matrices for the final fused transpose+shift matmuls
    idb_d = nc.dram_tensor("idb", [128, 128], bf16, kind="ExternalInput").ap()
    pdn_d = nc.dram_tensor("pdn", [128, 128], bf16, kind="ExternalInput").ap()
    pup_d = nc.dram_tensor("pup", [128, 128], bf16, kind="ExternalInput").ap()
    out_d = nc.dram_tensor("out", [DIM, SLAB, W], f32, kind="ExternalOutput").ap()
    if debug:
        dbg_y = nc.dram_tensor("dbg_y", [128, 36, 130], bf16,
                               kind="ExternalOutput").ap()
        dbg_ker = nc.dram_tensor("dbg_ker", [16, SLAB, W], bf16,
                                 kind="ExternalOutput").ap()
        dbg_kta = nc.dram_tensor("dbg_kta", [W, SLAB, 16], bf16,
                                 kind="ExternalOutput").ap()
        dbg_kt2 = nc.dram_tensor("dbg_kt2", [W, T, 2, 16, SLAB // 2], bf16,
                                 kind="ExternalOutput").ap()

    with TileContext(nc) as tc:
        with (
            tc.tile_pool(name="consts", bufs=1) as cpool,
            tc.tile_pool(name="xcp", bufs=2) as xcp,
            tc.tile_pool(name="xtp", bufs=3) as xtp,
            tc.tile_pool(name="yp", bufs=2) as yp,
            tc.tile_pool(name="kerp", bufs=2) as kerp,
            tc.tile_pool(name="ktap", bufs=2) as ktap,
            tc.tile_pool(name="ktp", bufs=1) as ktp,
            tc.tile_pool(name="accp", bufs=1) as accp,
            tc.tile_pool(name="stage", bufs=6) as stp,
            tc.tile_pool(name="obp", bufs=3) as obp,
        ):
            # consts are issued on the scalar-engine DGE so the sync DGE can
            # start streaming frame 0's inputs immediately
            w1p_sb = cpool.tile([128, 3, DIM], bf16)
            nc.scalar.dma_start(out=w1p_sb, in_=w1p_d)
            w1q_sb = cpool.tile([128, DIM], bf16)
            nc.scalar.dma_start(out=w1q_sb, in_=w1q_d)
            w1s2_sb = cpool.tile([64, DIM], bf16)
            nc.scalar.dma_start(out=w1s2_sb, in_=w1s2_d)
            w2p_sb = cpool.tile([128, 3, 9], bf16)
            nc.scalar.dma_start(out=w2p_sb, in_=w2p_d)
            w2s_sb = cpool.tile([128, 3, 9], bf16)
            nc.scalar.dma_start(out=w2s_sb[64:128], in_=w2s_d)
            b1_sb = cpool.tile([DIM, 1], f32)
            nc.scalar.dma_start(out=b1_sb, in_=b1_d)
            b2_sb = cpool.tile([9, 1], f32)
            nc.scalar.dma_start(out=b2_sb, in_=b2_d)
            ym_sb = cpool.tile([128, 2], f32)
            nc.scalar.dma_start(out=ym_sb, in_=ym_d)
            em_sb = cpool.tile([W, 1], f32)
            nc.gpsimd.dma_start(out=em_sb, in_=em_d)
            ef_sb = cpool.tile([W, 1], f32)
            nc.gpsimd.dma_start(out=ef_sb, in_=ef_d)
            ea_sb = cpool.tile([W, 1], f32)
            nc.gpsimd.dma_start(out=ea_sb, in_=ea_d)
            eb_sb = cpool.tile([W, 1], f32)
            nc.gpsimd.dma_start(out=eb_sb, in_=eb_d)
            idb_sb = cpool.tile([128, 128], bf16)
            nc.gpsimd.dma_start(out=idb_sb, in_=idb_d)
            pdn_sb = cpool.tile([128, 128], bf16)
            nc.gpsimd.dma_start(out=pdn_sb, in_=pdn_d)
            pup_sb = cpool.tile([128, 128], bf16)
            nc.gpsimd.dma_start(out=pup_sb, in_=pup_d)
            al_sb = cpool.tile([DIM, 1], f32)
            nc.vector.memset(al_sb, 0.2)

            # per-pixel kernels, [q, frame, r-half, tap16, r16] bf16
            # (taps 9..15 unused; r-half-major so per-half DMAs are
            # contiguous)
            kt2 = ktp.tile([W, T, 2, 16, SLAB // 2], bf16)
            kt_p1 = ktp.tile([W, T, 2, 16, SLAB // 2], bf16)
            kt_m1 = ktp.tile([W, T, 2, 16, SLAB // 2], bf16)
            nc.gpsimd.memset(kt_p1[96:128], 0.0)
            nc.gpsimd.memset(kt_m1[0:32], 0.0)
            sum45 = ktp.tile([W, SLAB], f32)
            t45 = ktp.tile([W, SLAB], f32)

            # bf16 accumulators, one per dj; [q, c, r]
            accs = [accp.tile([W, DIM, SLAB], bf16, name=f"acc{dj}")
                    for dj in range(3)]
            ksrc = [kt_p1, kt2, kt_m1]
            u_sb = accp.tile([W, DIM, 34], bf16)

            def emit_loads(f):
                """Prefetch frame f's inputs; the sync DGE queue carries
                only these bulk loads so they stream FIFO ahead of use."""
                xp = xcp.tile([128, 36, 130], bf16, tag="xp")
                nc.sync.dma_start(out=xp, in_=xp_d[:, f])
                xq = xcp.tile([128, 36, 130], bf16, tag="xq")
                nc.sync.dma_start(out=xq, in_=xq_d[:, f])
                xt_f = xtp.tile([W, DIM, 34], bf16, tag="xt")
                nc.sync.dma_start(out=xt_f, in_=xt_d[:, f])
                return xp, xq, xt_f

            def emit_conv1(f, loads, ps1p):
                xp, xq, xt_f = loads
                y2 = yp.tile([128, 36, 130], bf16, tag="y2")
                nc.gpsimd.memset(y2[:, :, 0:1], 0.0)
                nc.gpsimd.memset(y2[:, :, 129:130], 0.0)
                if f == 0:
                    nc.scalar.activation(u_sb, xt_f, Act.Copy, scale=1.0)
                else:
                    nc.gpsimd.tensor_tensor(u_sb, u_sb, xt_f, Alu.add)

                for rc in range(9):
                    g0 = 1 + 4 * rc
                    nr = 4 if rc < 8 else 2
                    ps = ps1p.tile([DIM, 4, W], f32, tag="ps1")
                    for i, dj in enumerate(range(3)):
                        nc.tensor.matmul(
                            ps[:, :nr, :],
                            lhsT=w1p_sb[:, dj, :],
                            rhs=xp[:, g0 - 1:g0 - 1 + nr, dj:dj + W],
                            start=(i == 0),
                            stop=False,
                        )
                    nc.tensor.matmul(
                        ps[:, :nr, :],
                        lhsT=w1q_sb,
                        rhs=xq[:, g0:g0 + nr, 0:W],
                        start=False,
                        stop=False,
                    )
                    nc.tensor.matmul(
                        ps[:, :nr, :],
                        lhsT=w1s2_sb,
                        rhs=xq[0:64, g0:g0 + nr, 2:2 + W],
                        start=False,
                        stop=True,
                    )
                    nc.scalar.activation(y2[0:64, g0:g0 + nr, 1:129],
                                         ps[:, :nr], Act.Prelu,
                                         bias=b1_sb, scale=1.0, alpha=al_sb)
                    if rc == 0:
                        # conv2 zero-pads rows outside the image: scale the
                        # y rows that fall outside (mask is 0 on edge slabs)
                        nc.scalar.activation(y2[0:64, 1:2, 1:129],
                                             y2[0:64, 1:2, 1:129],
                                             Act.Copy, scale=ym_sb[0:64, 0:1])
                    if rc == 4:
                        # early half of the row-shifted copy for tap pairing
                        nc.gpsimd.dma_start(out=y2[64:128, 0:18],
                                            in_=y2[0:64, 1:19])
                nc.scalar.activation(y2[0:64, 34:35, 1:129],
                                     y2[0:64, 34:35, 1:129],
                                     Act.Copy, scale=ym_sb[0:64, 1:2])
                nc.gpsimd.dma_start(out=y2[64:128, 18:35],
                                    in_=y2[0:64, 19:36])
                if debug and f == 0:
                    nc.sync.dma_start(out=dbg_y, in_=y2)
                return y2

            HS = SLAB // 2  # half-slab rows

            def emit_conv2_half(f, h, y2, ps2p):
                """conv2 for output rows h*16..h*16+16, ker transpose and
                kernel staging for that half."""
                ker16 = kerp.tile([16, HS, W], bf16, tag="ker16")
                for rc4 in range(4):
                    rc = 4 * h + rc4
                    c0 = 2 + 4 * rc
                    ps2 = ps2p.tile([9, 4, W], f32, tag="ps2")
                    for i, dj in enumerate(range(3)):
                        nc.tensor.matmul(
                            ps2,
                            lhsT=w2p_sb[:, dj, :],
                            rhs=y2[:, c0 - 1:c0 + 3, dj:dj + W],
                            start=(i == 0),
                            stop=False,
                        )
                    for i, dj in enumerate(range(3)):
                        nc.tensor.matmul(
                            ps2,
                            lhsT=w2s_sb[64:128, dj, :],
                            rhs=y2[64:128, c0:c0 + 4, dj:dj + W],
                            start=False,
                            stop=(i == 2),
                        )
                    nc.scalar.activation(
                        ker16[0:9, 4 * rc4:4 * rc4 + 4, :],
                        ps2, Act.Identity, bias=b2_sb, scale=1.0)

                # transpose (tap, r, q) -> (q, r, tap) via the DMA XBAR
                # (xbar block b of 128 cols lands at out[:, b, :]),
                # then repack to (q, tap, r) so filtering reads are
                # innermost-contiguous (enables the DVE 16-bit 2x mode)
                rh = slice(HS * h, HS * h + HS)
                kt_a = ktap.tile([W, HS, 16], bf16, tag="kta")
                nc.scalar.dma_start_transpose(
                    out=kt_a, in_=ker16.rearrange("k r q -> k (r q)"))
                nc.vector.tensor_copy(kt2[:, f, h],
                                      kt_a.rearrange("q r t -> q t r"))
                if debug and f == 0:
                    nc.sync.dma_start(out=dbg_ker[:, rh], in_=ker16)
                    nc.sync.dma_start(out=dbg_kta[:, rh], in_=kt_a)

                # fold W-edge replicate-pad terms into the dj=1 slot
                ktr = kt2[:, f, h, 0:9, :].rearrange(
                    "q (di dj) r -> q di dj r", di=3, dj=3)
                nc.vector.tensor_tensor(ktr[0:1, :, 1, :], ktr[0:1, :, 1, :],
                                        ktr[0:1, :, 0, :], Alu.add)
                nc.vector.scalar_tensor_tensor(
                    out=ktr[96:128, :, 1, :],
                    in0=ktr[96:128, :, 2, :], scalar=em_sb[96:128, :],
                    in1=ktr[96:128, :, 1, :], op0=Alu.mult, op1=Alu.add)

                # partition-shifted kernel copies for the dj column shifts
                nc.gpsimd.dma_start(out=kt_p1[0:127, f, h],
                                    in_=kt2[1:128, f, h])
                nc.gpsimd.dma_start(out=kt_m1[1:128, f, h],
                                    in_=kt2[0:127, f, h])

                # incremental sum of the 45 (folded) kernel taps
                t_out = sum45 if f == 0 else t45
                nc.vector.tensor_reduce(
                    t_out[:, rh],
                    kt2[:, f, h, 0:9, :].rearrange("q t r -> q r t"),
                    axis=mybir.AxisListType.X, op=Alu.add)
                if f > 0:
                    nc.vector.tensor_tensor(sum45[:, rh], sum45[:, rh],
                                            t45[:, rh], Alu.add)

            def _acc_engine(dj):
                return nc.vector

            def emit_filter_half(f, h, xt_f):
                rh = slice(HS * h, HS * h + HS)
                for dj in range(3):
                    eng = _acc_engine(dj)
                    prods = []
                    for di in range(3):
                        kb = ksrc[dj][:, f, h, 3 * di + dj, :].unsqueeze(1)\
                            .broadcast_to((W, DIM, HS))
                        prod = stp.tile([W, DIM, HS], bf16, tag="prod")
                        nc.vector.tensor_tensor(
                            prod, xt_f[:, :, HS * h + di:HS * h + di + HS],
                            kb, Alu.mult)
                        prods.append(prod)
                    a = accs[dj][:, :, rh]
                    if f == 0:
                        eng.tensor_tensor(a, prods[0], prods[1], Alu.add)
                    else:
                        eng.tensor_tensor(a, a, prods[0], Alu.add)
                        eng.tensor_tensor(a, a, prods[1], Alu.add)
                    eng.tensor_tensor(a, a, prods[2], Alu.add)

            def emit_filter_full(f, xt_f):
                # full-slab filtering: the kernel operand spans both r-halves
                # via a 4D view (innermost r stays packed for the 2x mode)
                for dj in range(3):
                    eng = _acc_engine(dj)
                    prods = []
                    for di in range(3):
                        kb = ksrc[dj][:, f, :, 3 * di + dj, :].unsqueeze(1)\
                            .broadcast_to((W, DIM, 2, HS))
                        prod = stp.tile([W, DIM, SLAB], bf16, tag="prod")
                        pv = prod.rearrange("q c (h r) -> q c h r", h=2)
                        xv = xt_f[:, :, di:di + SLAB]\
                            .rearrange("q c (h r) -> q c h r", h=2)
                        nc.vector.tensor_tensor(pv, xv, kb, Alu.mult)
                        prods.append(prod)
                    a = accs[dj]
                    if f == 0:
                        eng.tensor_tensor(a, prods[0], prods[1], Alu.add)
                    else:
                        eng.tensor_tensor(a, a, prods[0], Alu.add)
                        eng.tensor_tensor(a, a, prods[1], Alu.add)
                    eng.tensor_tensor(a, a, prods[2], Alu.add)

            c_sb = ktp.tile([W, SLAB], f32)
            corr = ktp.tile([W, SLAB], f32)
            c_p1 = ktp.tile([W, SLAB], f32)
            c_m1 = ktp.tile([W, SLAB], f32)
            c_c = ktp.tile([W, SLAB], f32)
            cb_p1 = ktp.tile([W, SLAB], bf16)
            cb_c = ktp.tile([W, SLAB], bf16)
            cb_m1 = ktp.tile([W, SLAB], bf16)
            nc.gpsimd.memset(c_p1[96:128], 0.0)
            nc.gpsimd.memset(c_m1[0:32], 0.0)
            s_sb = accp.tile([W, DIM, SLAB], bf16)

            def emit_S():
                # S = 3-row vertical box of U (edge rows clamped in xt);
                # on gpsimd, in parallel with the last frame's filtering
                nc.gpsimd.tensor_tensor(s_sb, u_sb[:, :, 0:SLAB],
                                        u_sb[:, :, 1:SLAB + 1], Alu.add)
                nc.gpsimd.tensor_tensor(s_sb, s_sb, u_sb[:, :, 2:SLAB + 2],
                                        Alu.add)

            def emit_c_chain():
                # c = 1/45 - mean(ker); sum45 reads the folded kernel, so
                # undo the fold's double-count at the edge partitions.
                nc.vector.tensor_scalar(c_sb, sum45, -1.0 / 45.0, 1.0 / 45.0,
                                        Alu.mult, Alu.add)
                for h in range(2):
                    rh = slice(HS * h, HS * h + HS)
                    ktr_r = kt2[:, :, h, 0:9, :].rearrange(
                        "q f (di dj) r -> q r f di dj", di=3, dj=3)
                    nc.vector.tensor_reduce(corr[0:32, rh],
                                            ktr_r[0:32, :, :, :, 0],
                                            axis=mybir.AxisListType.XY,
                                            op=Alu.add)
                    nc.vector.tensor_reduce(corr[96:128, rh],
                                            ktr_r[96:128, :, :, :, 2],
                                            axis=mybir.AxisListType.XY,
                                            op=Alu.add)
                nc.vector.scalar_tensor_tensor(
                    out=c_sb[0:32], in0=corr[0:32], scalar=ea_sb[0:32],
                    in1=c_sb[0:32], op0=Alu.mult, op1=Alu.add)
                nc.vector.scalar_tensor_tensor(
                    out=c_sb[96:128], in0=corr[96:128], scalar=eb_sb[96:128],
                    in1=c_sb[96:128], op0=Alu.mult, op1=Alu.add)
                # shifted + edge-doubled variants of c, bf16 for 2x filtering
                nc.sync.dma_start(out=c_p1[0:127], in_=c_sb[1:128])
                nc.sync.dma_start(out=c_m1[1:128], in_=c_sb[0:127])
                nc.vector.tensor_scalar(c_c, c_sb, ef_sb, None, Alu.mult)
                nc.gpsimd.tensor_copy(cb_p1, c_p1)
                nc.gpsimd.tensor_copy(cb_c, c_c)
                nc.gpsimd.tensor_copy(cb_m1, c_m1)

            # contiguous per-half staging of the final accumulators (the
            # permute matmul lhsT must have one free dim); written by the
            # cS add, so this costs no extra ops
            accsH = [[accp.tile([W, DIM, HS], bf16, name=f"accH{h}{dj}")
                      for dj in range(3)] for h in range(2)]

            def emit_cs_half(h):
                rh = slice(HS * h, HS * h + HS)
                for dj, csrc in ((0, cb_p1), (1, cb_c), (2, cb_m1)):
                    cbb = csrc[:, rh].unsqueeze(1).broadcast_to((W, DIM, HS))
                    prod = stp.tile([W, DIM, HS], bf16, tag="prod")
                    nc.vector.tensor_tensor(prod, s_sb[:, :, rh], cbb,
                                            Alu.mult)
                    nc.vector.tensor_tensor(accsH[h][dj], accs[dj][:, :, rh],
                                            prod, Alu.add)

            def emit_permute_half(h, psop):
                # fused transpose + dj merge via PSUM-accumulating permute
                # matmuls: out[m,p] = acc1[p,m] + acc0[p-1,m] + acc2[p+1,m]
                rh = slice(HS * h, HS * h + HS)
                af = [a.rearrange("q c r -> q (c r)") for a in accsH[h]]
                for b in range(8):
                    cs = slice(128 * b, 128 * b + 128)
                    l1 = af[1][:, cs]
                    l0 = af[0][:, cs]
                    l2 = af[2][:, cs]
                    pso = psop.tile([128, 128], f32, tag="pso")
                    nc.tensor.matmul(pso, lhsT=l1, rhs=idb_sb,
                                     start=True, stop=False)
                    nc.tensor.matmul(pso, lhsT=l0, rhs=pdn_sb,
                                     start=False, stop=False)
                    nc.tensor.matmul(pso, lhsT=l2, rhs=pup_sb,
                                     start=False, stop=True)
                    ob = obp.tile([128, 128], f32, tag="ob")
                    nc.scalar.activation(ob, pso, Act.Copy, scale=1.0)
                    eng = nc.sync if b % 2 == 0 else nc.scalar
                    eng.dma_start(out=out_d[8 * b:8 * b + 8, rh], in_=ob)

            with (
                tc.tile_pool(name="ps1", bufs=3, space="PSUM") as ps1p,
                tc.tile_pool(name="ps2", bufs=3, space="PSUM") as ps2p,
                tc.tile_pool(name="pso", bufs=2, space="PSUM") as psop,
            ):
                loads = emit_loads(0)
                for f in range(T):
                    nxt = emit_loads(f + 1) if f + 1 < T else None
                    xt_f = loads[2]
                    y2 = emit_conv1(f, loads, ps1p)
                    if f < T - 2:
                        emit_conv2_half(f, 0, y2, ps2p)
                        emit_conv2_half(f, 1, y2, ps2p)
                        emit_filter_full(f, xt_f)
                    elif f == T - 2:
                        # half-split the second-to-last frame too, so the
                        # DVE is drained when the last conv finishes
                        emit_conv2_half(f, 0, y2, ps2p)
                        emit_filter_half(f, 0, xt_f)
                        emit_conv2_half(f, 1, y2, ps2p)
                        emit_filter_half(f, 1, xt_f)
                    else:
                        emit_S()
                        emit_conv2_half(f, 0, y2, ps2p)
                        emit_filter_half(f, 0, xt_f)
                        emit_conv2_half(f, 1, y2, ps2p)
                        if debug:
                            nc.sync.dma_start(out=dbg_kt2, in_=kt2)
                        emit_c_chain()
                        emit_cs_half(0)
                        emit_permute_half(0, psop)
                        emit_filter_half(f, 1, xt_f)
                        emit_cs_half(1)
                        emit_permute_half(1, psop)
                    loads = nxt

    return nc


def _get_program():
    if "nc" not in _PROGRAM_CACHE:
        nc = _build_program()
        nc.finalize()
        _PROGRAM_CACHE["nc"] = nc
    return _PROGRAM_CACHE["nc"]


def _get_program_debug():
    if "ncd" not in _PROGRAM_CACHE:
        nc = _build_program(debug=True)
        nc.finalize()
        _PROGRAM_CACHE["ncd"] = nc
    return _PROGRAM_CACHE["ncd"]


def _host_prep(x, w1, b1, w2, b2):
    """Build the 8 per-core input maps from full inputs."""
    import ml_dtypes
    bf16 = ml_dtypes.bfloat16

    x = np.asarray(x, dtype=np.float32)
    w1 = np.asarray(w1, dtype=np.float32)
    b1 = np.asarray(b1, dtype=np.float32)
    w2 = np.asarray(w2, dtype=np.float32)
    b2 = np.asarray(b2, dtype=np.float32)

    # paired conv weights: [pairs di=0,1 stacked on K, then di=2 single]
    # w1p[ci, dj, o] = w1[o, ci, 0, dj]; w1p[64+ci, dj, o] = w1[o, ci, 1, dj]
    w1p = np.concatenate([w1[:, :, 0, :].transpose(1, 2, 0),
                          w1[:, :, 1, :].transpose(1, 2, 0)], axis=0)
    # w1q pairs taps (2,0)+(2,1) on a column-shifted rhs; w1s2 is tap (2,2)
    w1q = np.concatenate([w1[:, :, 2, 0].T, w1[:, :, 2, 1].T], axis=0)
    w1s2 = np.ascontiguousarray(w1[:, :, 2, 2].T)
    w2p = np.concatenate([w2[:, :, 0, :].transpose(1, 2, 0),
                          w2[:, :, 1, :].transpose(1, 2, 0)], axis=0)
    w2s = np.ascontiguousarray(w2[:, :, 2, :].transpose(1, 2, 0))

    b1c = np.ascontiguousarray(b1.reshape(DIM, 1))
    b2c = np.ascontiguousarray(b2.reshape(9, 1))
    idb = np.eye(128, dtype=np.float32)
    pdn = np.zeros((128, 128), dtype=np.float32)   # pdn[k, p]=1 iff k==p-1
    pdn[np.arange(127), np.arange(1, 128)] = 1.0
    pup = np.zeros((128, 128), dtype=np.float32)   # pup[k, p]=1 iff k==p+1
    pup[np.arange(1, 128), np.arange(127)] = 1.0
    emask = np.zeros((W, 1), dtype=np.float32)
    emask[127, 0] = 1.0
    efold = np.ones((W, 1), dtype=np.float32)
    efold[0, 0] = 2.0
    efold[127, 0] = 2.0
    emA = np.zeros((W, 1), dtype=np.float32)
    emA[0, 0] = 1.0 / 45.0
    emB = np.zeros((W, 1), dtype=np.float32)
    emB[127, 0] = 1.0 / 45.0

    w1p = w1p.astype(bf16)
    w1q = w1q.astype(bf16)
    w1s2 = w1s2.astype(bf16)
    w2p = w2p.astype(bf16)
    w2s = w2s.astype(bf16)
    idb = idb.astype(bf16)
    pdn = pdn.astype(bf16)
    pup = pup.astype(bf16)

    in_maps = []
    for core in range(NCORES):
        b, s = divmod(core, 4)
        r0 = s * SLAB
        # conv input: rows r0-2 .. r0+34 zero padded, cols -1..128 zero padded
        xc = np.zeros((DIM, T, 37, 130), dtype=np.float32)
        lo = max(0, r0 - 2)
        hi = min(H, r0 + 35)
        xc[:, :, lo - (r0 - 2):hi - (r0 - 2), 1:129] = x[b, :, :, lo:hi, :]
        # paired conv rhs tiles: lower half plain, upper half row-shifted
        # (xp) / row-shifted + col-shifted (xq)
        xp = np.concatenate([xc[:, :, 0:36], xc[:, :, 1:37]], axis=0)
        xq_hi = np.zeros((DIM, T, 36, 130), dtype=np.float32)
        xq_hi[:, :, :, 0:129] = xc[:, :, 1:37, 1:130]
        xq = np.concatenate([xc[:, :, 1:37], xq_hi], axis=0)
        # filter input, pixel-partition, innermost rows:
        # xt[q, t, c, j] = x[b, c, t, clip(r0-1+j), q]
        rows = np.clip(np.arange(r0 - 1, r0 + 33), 0, H - 1)
        xt = np.ascontiguousarray(x[b][:, :, rows, :].transpose(3, 1, 0, 2))
        # conv2 zero-pad mask for the y halo rows (y rows 1 and 34)
        ymask = np.ones((128, 2), dtype=np.float32)
        if s == 0:
            ymask[:, 0] = 0.0
        if s == 3:
            ymask[:, 1] = 0.0
        in_maps.append({
            "xp": xp.astype(bf16), "xq": xq.astype(bf16),
            "xt": xt.astype(bf16),
            "w1p": w1p, "w1q": w1q, "w1s2": w1s2, "w2p": w2p, "w2s": w2s,
            "b1c": b1c, "b2c": b2c, "ymask": ymask, "emask": emask,
            "efold": efold, "emA": emA, "emB": emB,
            "idb": idb, "pdn": pdn, "pup": pup,
        })
    return in_maps


def kernel(x, w1, b1, w2, b2):
    from concourse.bass_utils import run_bass_kernel_spmd

    nc = _get_program()
    in_maps = _host_prep(x, w1, b1, w2, b2)
    res = run_bass_kernel_spmd(nc, in_maps, list(range(NCORES)))
    out = np.zeros((2, DIM, H, W), dtype=np.float32)
    for core in range(NCORES):
        b, s = divmod(core, 4)
        out[b, :, s * SLAB:(s + 1) * SLAB, :] = res.results[core]["out"]
    return out
